# revision 4
# baseline (speedup 1.0000x reference)
"""Trainium2 Bass/Tile kernel: fused fp8-quantized multi-head causal attention.

Module: q/k/v = fp8(x) @ fp8(W) + b ; scores = (q k^T)/sqrt(64) with causal
mask (-1000 => exp underflows to exactly 0) ; out = softmax(scores) @ v @ W_O + b_O.

Sharding (8 NeuronCores, SPMD, no collectives):
  core c -> batch b = c // 4, head group hg = c % 4 (heads 4*hg .. 4*hg+3).
  Each core returns a partial [S, M] output (its 4 heads' contribution);
  the host sums the 4 partials per batch and adds b_O.

Host-side preprocessing: inputs/W_{Q,K,V} are quantized to fp8-e4m3 on the
host (bit-identical to the reference's jnp e4m3fn cast for |x| <= 240) and
activations are uploaded transposed [M, S] so the contraction dim lands on
SBUF partitions. W_O is uploaded as bf16.

On-chip layout / dataflow per core:
  qT, kT   : [d'=256, S]  (d' = 4 heads x 64), psum <- sum_m W^T x^T (+bias),
             q scaled by 1/8 (exact, exponent-only) at eviction -> bf16.
  v        : [S, 260]     ([s, head-major 4x(64+ones-col)]) -> bf16.
  scores^T : psum [sk=128, sq=512] = kT_h^T(lhsT) @ qT_h  (K=64)
  pattern  : exp on ScalarE (no max-subtraction needed; |scores| < ~3),
             causal-masked tiles multiplied by a precomputed 0/1 mask.
  z^T+denom: psum [65, sq] += v_h(lhsT [sk,65]) @ pattern  (ones col => row 64
             accumulates the softmax denominator for free).
  normalize: recip = 1/denom row; broadcast across 64 partitions via a K=1
             matmul with a ones column; zT_norm = zT * recip -> bf16.
  out      : psum [s=128, m=512] = zt(lhsT [hd=128,s])^T @ W_O, 2 hd chunks,
             evict fp32 -> DRAM partial.
"""

import os
import sys

for _p in ("/opt/trn_rl_repo", os.path.expanduser("~/.axon_site/_ro/trn_rl_repo")):
    if os.path.isdir(_p) and _p not in sys.path:
        sys.path.insert(0, _p)

import ml_dtypes
import numpy as np

import concourse.bass as bass
import concourse.mybir as mybir
import concourse.tile as tile
from concourse import bacc
from concourse.bass_utils import run_bass_kernel_spmd

B, S, M, H, D = 2, 2048, 1024, 16, 64
HG = 4                 # heads per core
NCORES = 8
SQ = 512               # sq chunk width (one fp32 psum bank)
NSQ = S // SQ          # 4
NMC = M // 128         # 8 contraction chunks for projections
NSS = S // 128         # 16 s sub-chunks of 128

F8 = mybir.dt.float8e4
BF = mybir.dt.bfloat16
F32 = mybir.dt.float32
EXP = mybir.ActivationFunctionType.Exp

_f8 = ml_dtypes.float8_e4m3
_bf16 = ml_dtypes.bfloat16


def _build_nc():
    nc = bacc.Bacc(
        "TRN2", target_bir_lowering=False, debug=False, num_devices=NCORES
    )

    xq = nc.declare_dram_parameter("xq_t8", [M, S], F8, isOutput=False)
    xk = nc.declare_dram_parameter("xk_t8", [M, S], F8, isOutput=False)
    xv = nc.declare_dram_parameter("xv_t8", [M, S], F8, isOutput=False)
    wq = nc.declare_dram_parameter("wq8", [M, HG * D], F8, isOutput=False)
    wk = nc.declare_dram_parameter("wk8", [M, HG * D], F8, isOutput=False)
    wv = nc.declare_dram_parameter("wv8", [M, HG * D], F8, isOutput=False)
    wo = nc.declare_dram_parameter("wo_bf", [HG * D, M], BF, isOutput=False)
    bq = nc.declare_dram_parameter("bq", [1, HG * D], F32, isOutput=False)
    bk = nc.declare_dram_parameter("bk", [1, HG * D], F32, isOutput=False)
    bv = nc.declare_dram_parameter("bv", [1, HG * D], F32, isOutput=False)
    out_p = nc.declare_dram_parameter("out_p", [S, M], F32, isOutput=True)

    with tile.TileContext(nc) as tc:
        with (
            tc.tile_pool(name="persist", bufs=1) as pers,
            tc.tile_pool(name="work", bufs=3) as work,
            tc.tile_pool(name="ppa", bufs=2, space="PSUM") as ppa,
            tc.tile_pool(name="pps", bufs=2, space="PSUM") as pps,
            tc.tile_pool(name="ppz", bufs=2, space="PSUM") as ppz,
        ):
            # ---- persistent SBUF tensors ----
            xq_sb = pers.tile([128, NMC, S], F8, tag="xq")
            xk_sb = pers.tile([128, NMC, S], F8, tag="xk")
            xv_sb = pers.tile([128, NMC, S], F8, tag="xv")
            wq_sb = pers.tile([128, NMC, HG * D], F8, tag="wq")
            wk_sb = pers.tile([128, NMC, HG * D], F8, tag="wk")
            wv_sb = pers.tile([128, NMC, HG * D], F8, tag="wv")
            wo_sb = pers.tile([128, 2, M], BF, tag="wo")
            bq_sb = pers.tile([1, HG * D], F32, tag="bq")
            bk_sb = pers.tile([1, HG * D], F32, tag="bk")
            bv_sb = pers.tile([1, HG * D], F32, tag="bv")
            qt_sb = pers.tile([128, 2, S], BF, tag="qt")
            kt_sb = pers.tile([128, 2, S], BF, tag="kt")
            zt_sb = pers.tile([128, 2, S], BF, tag="zt")
            v_sb = pers.tile([128, NSS, HG, D + 1], BF, tag="v")
            masks = pers.tile([128, 4, SQ], F32, tag="masks")
            ones = pers.tile([1, SQ], F32, tag="ones")

            # ---- constants ----
            nc.gpsimd.memset(ones[:, :], 1.0)
            nc.gpsimd.memset(v_sb[:, :, :, D : D + 1], 1.0)
            nc.gpsimd.memset(masks[:, :, :], 1.0)
            for r in range(4):
                # keep 1.0 where (row + 128*r) <= col, else 0.0
                nc.gpsimd.affine_select(
                    out=masks[:, r, :],
                    in_=masks[:, r, :],
                    compare_op=mybir.AluOpType.is_ge,
                    fill=0.0,
                    base=-128 * r,
                    pattern=[[1, SQ]],
                    channel_multiplier=-1,
                )

            # ---- input DMAs ----
            for mi in range(NMC):
                sl = slice(128 * mi, 128 * mi + 128)
                nc.sync.dma_start(out=xq_sb[:, mi, :], in_=xq[sl, :])
                nc.sync.dma_start(out=xk_sb[:, mi, :], in_=xk[sl, :])
                nc.sync.dma_start(out=xv_sb[:, mi, :], in_=xv[sl, :])
            nc.sync.dma_start(
                out=wq_sb[:, :, :], in_=wq[:, :].rearrange("(c p) d -> p c d", p=128)
            )
            nc.sync.dma_start(
                out=wk_sb[:, :, :], in_=wk[:, :].rearrange("(c p) d -> p c d", p=128)
            )
            nc.sync.dma_start(
                out=wv_sb[:, :, :], in_=wv[:, :].rearrange("(c p) d -> p c d", p=128)
            )
            for c in range(2):
                nc.sync.dma_start(out=wo_sb[:, c, :], in_=wo[128 * c : 128 * c + 128, :])
            nc.sync.dma_start(out=bq_sb[:, :], in_=bq[:, :])
            nc.sync.dma_start(out=bk_sb[:, :], in_=bk[:, :])
            nc.sync.dma_start(out=bv_sb[:, :], in_=bv[:, :])

            # ---- phase 1a: qT / kT projections ([d', s] layout) ----
            for t in range(NSQ):
                ssl = slice(SQ * t, SQ * t + SQ)
                for half in range(2):
                    dsl = slice(128 * half, 128 * half + 128)
                    for dst_sb, w_sb, x_sb, b_sb, scale in (
                        (qt_sb, wq_sb, xq_sb, bq_sb, 0.125),
                        (kt_sb, wk_sb, xk_sb, bk_sb, None),
                    ):
                        ps = ppa.tile([128, SQ], F32, tag="ppa")
                        for mi in range(NMC):
                            nc.tensor.matmul(
                                ps[:, :],
                                lhsT=w_sb[:, mi, dsl],
                                rhs=x_sb[:, mi, ssl],
                                start=(mi == 0),
                                stop=False,
                            )
                        nc.tensor.matmul(
                            ps[:, :],
                            lhsT=b_sb[0:1, dsl],
                            rhs=ones[0:1, :],
                            start=False,
                            stop=True,
                        )
                        if scale is None:
                            nc.vector.tensor_copy(dst_sb[:, half, ssl], ps[:, :])
                        else:
                            nc.vector.tensor_scalar_mul(
                                dst_sb[:, half, ssl], ps[:, :], scale
                            )

            # ---- phase 1b: v projection ([s, d'] layout, ones col interleaved) ----
            for ss in range(NSS):
                psl = slice(128 * ss, 128 * ss + 128)
                ps = ppa.tile([128, SQ], F32, tag="ppa")
                for mi in range(NMC):
                    nc.tensor.matmul(
                        ps[:, 0 : HG * D],
                        lhsT=xv_sb[:, mi, psl],
                        rhs=wv_sb[:, mi, :],
                        start=(mi == 0),
                        stop=False,
                    )
                nc.tensor.matmul(
                    ps[:, 0 : HG * D],
                    lhsT=ones[0:1, 0:128],
                    rhs=bv_sb[0:1, :],
                    start=False,
                    stop=True,
                )
                nc.vector.tensor_copy(
                    v_sb[:, ss, :, 0:D],
                    ps[:, 0 : HG * D].rearrange("p (g d) -> p g d", g=HG),
                )

            # ---- phase 2: attention ----
            for jq in range(NSQ):
                qsl = slice(SQ * jq, SQ * jq + SQ)
                nsk = 4 * (jq + 1)
                for h in range(HG):
                    c, r0 = h // 2, 64 * (h % 2)
                    hsl = slice(r0, r0 + 64)
                    ps_z = ppz.tile([128, SQ], F32, tag="ppz")
                    for si in range(nsk):
                        ksl = slice(128 * si, 128 * si + 128)
                        ps_s = pps.tile([128, SQ], F32, tag="pps")
                        nc.tensor.matmul(
                            ps_s[:, :],
                            lhsT=kt_sb[hsl, c, ksl],
                            rhs=qt_sb[hsl, c, qsl],
                            start=True,
                            stop=True,
                        )
                        p_bf = work.tile([128, SQ], BF, tag="p")
                        nc.scalar.activation(p_bf[:, :], ps_s[:, :], EXP)
                        r = si - 4 * jq
                        if 0 <= r <= 3:
                            nc.vector.tensor_mul(p_bf[:, :], p_bf[:, :], masks[:, r, :])
                        nc.tensor.matmul(
                            ps_z[0 : D + 1, :],
                            lhsT=v_sb[:, si, h, :],
                            rhs=p_bf[:, :],
                            start=(si == 0),
                            stop=(si == nsk - 1),
                        )
                    # normalize: zT[0:64] * (1/denom row 64), broadcast via matmul
                    recip = work.tile([1, SQ], F32, tag="recip")
                    nc.vector.reciprocal(recip[:, :], ps_z[D : D + 1, :])
                    ps_b = pps.tile([128, SQ], F32, tag="pps")
                    nc.tensor.matmul(
                        ps_b[0:D, :],
                        lhsT=ones[0:1, 0:D],
                        rhs=recip[:, :],
                        start=True,
                        stop=True,
                    )
                    rb = work.tile([D, SQ], F32, tag="rb")
                    nc.vector.tensor_copy(rb[:, :], ps_b[0:D, :])
                    nc.vector.tensor_mul(zt_sb[hsl, c, qsl], ps_z[0:D, :], rb[:, :])

            # ---- phase 3: output projection ----
            for ss in range(NSS):
                psl = slice(128 * ss, 128 * ss + 128)
                for n in range(M // SQ):
                    nsl = slice(SQ * n, SQ * n + SQ)
                    ps = ppa.tile([128, SQ], F32, tag="ppa")
                    for c in range(2):
                        nc.tensor.matmul(
                            ps[:, :],
                            lhsT=zt_sb[:, c, psl],
                            rhs=wo_sb[:, c, nsl],
                            start=(c == 0),
                            stop=(c == 1),
                        )
                    o_sb = work.tile([128, SQ], F32, tag="o")
                    nc.vector.tensor_copy(o_sb[:, :], ps[:, :])
                    nc.sync.dma_start(out=out_p[psl, nsl], in_=o_sb[:, :])

    if not nc.is_finalized():
        nc.finalize()
    return nc


_NC = None


def _get_nc():
    global _NC
    if _NC is None:
        _NC = _build_nc()
    return _NC


def _make_in_maps(inputs):
    q8 = lambda a: np.asarray(a, np.float32).astype(_f8)
    xt = {}
    for name, key in (("xq_t8", "query_input"), ("xk_t8", "key_input"),
                      ("xv_t8", "value_input")):
        xt[name] = [np.ascontiguousarray(q8(inputs[key][b]).T) for b in range(B)]

    wq8 = q8(inputs["W_Q"])  # [H, M, D]
    wk8 = q8(inputs["W_K"])
    wv8 = q8(inputs["W_V"])
    wo = np.asarray(inputs["W_O"], np.float32)  # [H, D, M]

    in_maps = []
    for core in range(NCORES):
        b, hg = core // HG, core % HG
        hs = slice(HG * hg, HG * hg + HG)
        m = {
            "xq_t8": xt["xq_t8"][b],
            "xk_t8": xt["xk_t8"][b],
            "xv_t8": xt["xv_t8"][b],
            "wq8": np.ascontiguousarray(wq8[hs].transpose(1, 0, 2).reshape(M, HG * D)),
            "wk8": np.ascontiguousarray(wk8[hs].transpose(1, 0, 2).reshape(M, HG * D)),
            "wv8": np.ascontiguousarray(wv8[hs].transpose(1, 0, 2).reshape(M, HG * D)),
            "wo_bf": np.ascontiguousarray(wo[hs].reshape(HG * D, M).astype(_bf16)),
            "bq": np.asarray(inputs["b_Q"], np.float32)[hs].reshape(1, HG * D).copy(),
            "bk": np.asarray(inputs["b_K"], np.float32)[hs].reshape(1, HG * D).copy(),
            "bv": np.asarray(inputs["b_V"], np.float32)[hs].reshape(1, HG * D).copy(),
        }
        in_maps.append(m)
    return in_maps


def _run(inputs, **kw):
    nc = _get_nc()
    in_maps = _make_in_maps(inputs)
    res = run_bass_kernel_spmd(nc, in_maps, list(range(NCORES)), **kw)
    out = np.zeros((B, S, M), np.float32)
    for core in range(NCORES):
        out[core // HG] += res.results[core]["out_p"]
    out += np.asarray(inputs["b_O"], np.float32)
    return out, res


def kernel(**inputs):
    out, _ = _run(inputs)
    return out


# revision 12
# speedup vs baseline: 1.0780x; 1.0780x over previous
"""Trainium2 Bass/Tile kernel: fused fp8-quantized multi-head causal attention.

Module: q/k/v = fp8(x) @ fp8(W) + b ; scores = (q k^T)/sqrt(64) with causal
mask (-1000 => exp underflows to exactly 0) ; out = softmax(scores) @ v @ W_O + b_O.

Sharding (8 NeuronCores, SPMD, no collectives):
  core c -> batch b = c // 4, head group hg = c % 4 (heads 4*hg .. 4*hg+3).
  Each core returns a partial [S, M] output (its 4 heads' contribution);
  the host sums the 4 partials per batch and adds b_O.

Host-side preprocessing: inputs/W_{Q,K,V} are quantized to fp8-e4m3 on the
host (bit-identical to the reference's jnp e4m3fn cast for |x| <= 240) and
activations are uploaded transposed [M, S] so the contraction dim lands on
SBUF partitions. W_O is uploaded as bf16.

On-chip layout / dataflow per core:
  qT, kT   : [d'=256, S]  (d' = 4 heads x 64), DoubleRow fp8 matmuls,
             bias folded into the DVE eviction (per-partition scalar),
             q scaled by 1/8 (exact, exponent-only) at eviction -> bf16.
  v        : [S, 4x(64+ones-col)] -> bf16 (ones col makes the z^T matmul
             accumulate the softmax denominator in row 64 for free).
  scores^T : psum [sk=128, sq=512] = kT_h(lhsT [64,128]) @ qT_h  (K=64).
             Heads are processed in even/odd pairs whose lhsT/rhs live at
             partition bases 0/64 -> row-group-packed concurrent matmuls.
  pattern  : exp on ScalarE; diagonal tiles get restricted-width exp, a
             gpsimd zero-fill of the fully-masked band and one [128,128]
             triangular bf16 mask multiply on the boundary band.
  z^T+denom: psum [65, sq] += v_h(lhsT [sk,65]) @ pattern.
  normalize: 1/denom via batched reciprocal_approx_accurate (~2 ULP) on the
             head-pair's two denom rows; broadcast across the 64 partitions
             with a K=1 ones matmul; zT_norm = zT * recip -> bf16.
  out      : psum [s=128, m=512] = zt(lhsT [hd=128,s])^T @ W_O, 2 hd chunks,
             evict fp32 -> DRAM partial.
"""

import os
import sys

for _p in ("/opt/trn_rl_repo", os.path.expanduser("~/.axon_site/_ro/trn_rl_repo")):
    if os.path.isdir(_p) and _p not in sys.path:
        sys.path.insert(0, _p)

import ml_dtypes
import numpy as np

import concourse.bass as bass
import concourse.mybir as mybir
import concourse.tile as tile
from concourse import bacc
from concourse.bass_utils import run_bass_kernel_spmd

B, S, M, H, D = 2, 2048, 1024, 16, 64
HG = 4                 # heads per core
NCORES = 8
SQ = 512               # sq chunk width (one fp32 psum bank)
NSQ = S // SQ          # 4
NMC = M // 128         # 8 contraction chunks for projections
NSS = S // 128         # 16 s sub-chunks of 128

F8 = mybir.dt.float8e4
BF = mybir.dt.bfloat16
F32 = mybir.dt.float32
EXP = mybir.ActivationFunctionType.Exp
DR = mybir.MatmulPerfMode.DoubleRow

_f8 = ml_dtypes.float8_e4m3
_bf16 = ml_dtypes.bfloat16


def _build_nc():
    nc = bacc.Bacc(
        "TRN2", target_bir_lowering=False, debug=False, num_devices=NCORES
    )

    xq = nc.declare_dram_parameter("xq_t8", [M, S], F8, isOutput=False)
    xk = nc.declare_dram_parameter("xk_t8", [M, S], F8, isOutput=False)
    xv = nc.declare_dram_parameter("xv_t8", [M, S], F8, isOutput=False)
    wq = nc.declare_dram_parameter("wq8", [M, HG * D], F8, isOutput=False)
    wk = nc.declare_dram_parameter("wk8", [M, HG * D], F8, isOutput=False)
    wv = nc.declare_dram_parameter("wv8", [M, HG * D], F8, isOutput=False)
    wo = nc.declare_dram_parameter("wo_bf", [HG * D, M], BF, isOutput=False)
    bq = nc.declare_dram_parameter("bq", [128, 2], F32, isOutput=False)
    bk = nc.declare_dram_parameter("bk", [128, 2], F32, isOutput=False)
    bv = nc.declare_dram_parameter("bv", [1, HG * D], F32, isOutput=False)
    out_p = nc.declare_dram_parameter("out_p", [S, M], F32, isOutput=True)

    with tile.TileContext(nc) as tc:
        with (
            tc.tile_pool(name="persist", bufs=1) as pers,
            tc.tile_pool(name="work", bufs=4) as work,
            tc.tile_pool(name="ppa", bufs=2, space="PSUM") as ppa,
            tc.tile_pool(name="pps", bufs=3, space="PSUM") as pps,
            tc.tile_pool(name="ppz", bufs=3, space="PSUM") as ppz,
        ):
            # ---- persistent SBUF tensors ----
            xq_sb = pers.tile([128, NMC, S], F8, tag="xq")
            xk_sb = pers.tile([128, NMC, S], F8, tag="xk")
            xv_sb = pers.tile([128, NMC, S], F8, tag="xv")
            wq_sb = pers.tile([128, NMC, HG * D], F8, tag="wq")
            wk_sb = pers.tile([128, NMC, HG * D], F8, tag="wk")
            wv_sb = pers.tile([128, NMC, HG * D], F8, tag="wv")
            wo_sb = pers.tile([128, 2, M], BF, tag="wo")
            bq_sb = pers.tile([128, 2], F32, tag="bq")
            bk_sb = pers.tile([128, 2], F32, tag="bk")
            bv_sb = pers.tile([1, HG * D], F32, tag="bv")
            qt_sb = pers.tile([128, 2, S], BF, tag="qt")
            kt_sb = pers.tile([128, 2, S], BF, tag="kt")
            zt_sb = pers.tile([128, 2, S], BF, tag="zt")
            v_sb = pers.tile([128, NSS, HG, D + 1], BF, tag="v")
            trimask = pers.tile([128, 128], BF, tag="trimask")
            ones = pers.tile([1, SQ], F32, tag="ones")

            # ---- constants ----
            nc.gpsimd.memset(ones[:, :], 1.0)
            nc.gpsimd.memset(v_sb[:, :, :, D : D + 1], 1.0)
            # lower-triangular (inclusive) band mask: keep where row <= col
            nc.gpsimd.memset(trimask[:, :], 1.0)
            nc.gpsimd.affine_select(
                out=trimask[:, :],
                in_=trimask[:, :],
                compare_op=mybir.AluOpType.is_ge,
                fill=0.0,
                base=0,
                pattern=[[1, 128]],
                channel_multiplier=-1,
            )

            # ---- input DMAs (x split by s-chunk so projections start early) ----
            for mi in range(NMC):
                sl = slice(128 * mi, 128 * mi + 128)
                for t in range(NSQ):
                    ssl = slice(SQ * t, SQ * t + SQ)
                    nc.sync.dma_start(out=xq_sb[:, mi, ssl], in_=xq[sl, ssl])
                    nc.sync.dma_start(out=xk_sb[:, mi, ssl], in_=xk[sl, ssl])
                    nc.sync.dma_start(out=xv_sb[:, mi, ssl], in_=xv[sl, ssl])
            nc.sync.dma_start(
                out=wq_sb[:, :, :], in_=wq[:, :].rearrange("(c p) d -> p c d", p=128)
            )
            nc.sync.dma_start(
                out=wk_sb[:, :, :], in_=wk[:, :].rearrange("(c p) d -> p c d", p=128)
            )
            nc.sync.dma_start(
                out=wv_sb[:, :, :], in_=wv[:, :].rearrange("(c p) d -> p c d", p=128)
            )
            for c in range(2):
                nc.sync.dma_start(out=wo_sb[:, c, :], in_=wo[128 * c : 128 * c + 128, :])
            nc.sync.dma_start(out=bq_sb[:, :], in_=bq[:, :])
            nc.sync.dma_start(out=bk_sb[:, :], in_=bk[:, :])
            nc.sync.dma_start(out=bv_sb[:, :], in_=bv[:, :])

            # ---- phase 1a: qT / kT projections ([d', s], fp8 DoubleRow) ----
            for t in range(NSQ):
                ssl = slice(SQ * t, SQ * t + SQ)
                for half in range(2):
                    dsl = slice(128 * half, 128 * half + 128)
                    for dst_sb, w_sb, x_sb, b_sb, scale in (
                        (qt_sb, wq_sb, xq_sb, bq_sb, 0.125),
                        (kt_sb, wk_sb, xk_sb, bk_sb, None),
                    ):
                        ps = ppa.tile([128, SQ], F32, tag="ppa")
                        for mi in range(0, NMC, 2):
                            nc.tensor.matmul(
                                ps[:, :],
                                lhsT=w_sb[:, mi : mi + 2, dsl],
                                rhs=x_sb[:, mi : mi + 2, ssl],
                                start=(mi == 0),
                                stop=(mi == NMC - 2),
                                perf_mode=DR,
                            )
                        if scale is None:
                            nc.vector.tensor_scalar_add(
                                dst_sb[:, half, ssl], ps[:, :], b_sb[:, half : half + 1]
                            )
                        else:
                            nc.vector.tensor_scalar(
                                out=dst_sb[:, half, ssl],
                                in0=ps[:, :],
                                scalar1=b_sb[:, half : half + 1],
                                scalar2=scale,
                                op0=mybir.AluOpType.add,
                                op1=mybir.AluOpType.mult,
                            )

            # ---- phase 1b: v projection ([s, d'], ones col interleaved) ----
            for ss in range(NSS):
                psl = slice(128 * ss, 128 * ss + 128)
                ps = ppa.tile([128, SQ], F32, tag="ppa")
                for mi in range(0, NMC, 2):
                    nc.tensor.matmul(
                        ps[:, 0 : HG * D],
                        lhsT=xv_sb[:, mi : mi + 2, psl],
                        rhs=wv_sb[:, mi : mi + 2, :],
                        start=(mi == 0),
                        stop=False,
                        perf_mode=DR,
                    )
                nc.tensor.matmul(
                    ps[:, 0 : HG * D],
                    lhsT=ones[0:1, 0:128],
                    rhs=bv_sb[0:1, :],
                    start=False,
                    stop=True,
                )
                nc.vector.tensor_copy(
                    v_sb[:, ss, :, 0:D],
                    ps[:, 0 : HG * D].rearrange("p (g d) -> p g d", g=HG),
                )

            # ---- phase 2: attention, even/odd head pairs (row-group packed) ----
            for jq in range(NSQ):
                qsl = slice(SQ * jq, SQ * jq + SQ)
                nsk = 4 * (jq + 1)
                for c in range(2):  # head pair: heads (2c, 2c+1)
                    ps_z = [
                        ppz.tile([D + 1, SQ], F32, tag="ppz", name=f"psz{jq}_{c}_{u}")
                        for u in range(2)
                    ]
                    p_tiles = {}
                    for si in range(nsk):
                        ksl = slice(128 * si, 128 * si + 128)
                        r = si - 4 * jq  # >=0 on diagonal-band tiles
                        for u in range(2):  # head 2c+u at partition base 64*u
                            hsl = slice(64 * u, 64 * u + 64)
                            ps_s = pps.tile([128, SQ], F32, tag="pps")
                            nc.tensor.matmul(
                                ps_s[:, :],
                                lhsT=kt_sb[hsl, c, ksl],
                                rhs=qt_sb[hsl, c, qsl],
                                start=True,
                                stop=True,
                            )
                            p_bf = work.tile([128, SQ], BF, tag="p")
                            if r <= 0:
                                nc.scalar.activation(p_bf[:, :], ps_s[:, :], EXP)
                            else:
                                w0 = 128 * r
                                nc.gpsimd.memset(p_bf[:, 0:w0], 0.0)
                                nc.scalar.activation(
                                    p_bf[:, w0:SQ], ps_s[:, w0:SQ], EXP
                                )
                            if r >= 0:
                                w0 = 128 * r
                                nc.vector.tensor_mul(
                                    p_bf[:, w0 : w0 + 128],
                                    p_bf[:, w0 : w0 + 128],
                                    trimask[:, :],
                                )
                            p_tiles[u] = p_bf
                        for u in range(2):
                            h = 2 * c + u
                            nc.tensor.matmul(
                                ps_z[u][:, :],
                                lhsT=v_sb[:, si, h, :],
                                rhs=p_tiles[u][:, :],
                                start=(si == 0),
                                stop=(si == nsk - 1),
                            )
                    # normalize both heads of the pair
                    recip = work.tile([1, 2 * SQ], F32, tag="recip")
                    scratch = work.tile([1, 2 * SQ], F32, tag="rscr")
                    dn = work.tile([1, 2 * SQ], F32, tag="dn")
                    for u in range(2):
                        nc.vector.tensor_copy(
                            dn[0:1, SQ * u : SQ * u + SQ], ps_z[u][D : D + 1, :]
                        )
                    nc.vector.reciprocal_approx_accurate(
                        out=recip[:, :], in_=dn[:, :], scratch=scratch[:, :]
                    )
                    for u in range(2):
                        ps_b = pps.tile([128, SQ], F32, tag="pps")
                        nc.tensor.matmul(
                            ps_b[0:D, :],
                            lhsT=ones[0:1, 0:D],
                            rhs=recip[0:1, SQ * u : SQ * u + SQ],
                            start=True,
                            stop=True,
                        )
                        rb = work.tile([D, SQ], F32, tag="rb")
                        nc.vector.tensor_copy(rb[:, :], ps_b[0:D, :])
                        nc.vector.tensor_mul(
                            zt_sb[64 * u : 64 * u + 64, c, qsl],
                            ps_z[u][0:D, :],
                            rb[:, :],
                        )

            # ---- phase 3: output projection ----
            for ss in range(NSS):
                psl = slice(128 * ss, 128 * ss + 128)
                for n in range(M // SQ):
                    nsl = slice(SQ * n, SQ * n + SQ)
                    ps = ppa.tile([128, SQ], F32, tag="ppa")
                    for c in range(2):
                        nc.tensor.matmul(
                            ps[:, :],
                            lhsT=zt_sb[:, c, psl],
                            rhs=wo_sb[:, c, nsl],
                            start=(c == 0),
                            stop=(c == 1),
                        )
                    o_sb = work.tile([128, SQ], F32, tag="o")
                    nc.vector.tensor_copy(o_sb[:, :], ps[:, :])
                    nc.sync.dma_start(out=out_p[psl, nsl], in_=o_sb[:, :])

    if not nc.is_finalized():
        nc.finalize()
    return nc


_NC = None


def _get_nc():
    global _NC
    if _NC is None:
        _NC = _build_nc()
    return _NC


def _make_in_maps(inputs):
    q8 = lambda a: np.asarray(a, np.float32).astype(_f8)
    xt = {}
    for name, key in (("xq_t8", "query_input"), ("xk_t8", "key_input"),
                      ("xv_t8", "value_input")):
        xt[name] = [np.ascontiguousarray(q8(inputs[key][b]).T) for b in range(B)]

    wq8 = q8(inputs["W_Q"])  # [H, M, D]
    wk8 = q8(inputs["W_K"])
    wv8 = q8(inputs["W_V"])
    wo = np.asarray(inputs["W_O"], np.float32)  # [H, D, M]

    in_maps = []
    for core in range(NCORES):
        b, hg = core // HG, core % HG
        hs = slice(HG * hg, HG * hg + HG)
        m = {
            "xq_t8": xt["xq_t8"][b],
            "xk_t8": xt["xk_t8"][b],
            "xv_t8": xt["xv_t8"][b],
            "wq8": np.ascontiguousarray(wq8[hs].transpose(1, 0, 2).reshape(M, HG * D)),
            "wk8": np.ascontiguousarray(wk8[hs].transpose(1, 0, 2).reshape(M, HG * D)),
            "wv8": np.ascontiguousarray(wv8[hs].transpose(1, 0, 2).reshape(M, HG * D)),
            "wo_bf": np.ascontiguousarray(wo[hs].reshape(HG * D, M).astype(_bf16)),
            "bq": np.ascontiguousarray(np.asarray(inputs["b_Q"], np.float32)[hs].reshape(2, 128).T),
            "bk": np.ascontiguousarray(np.asarray(inputs["b_K"], np.float32)[hs].reshape(2, 128).T),
            "bv": np.asarray(inputs["b_V"], np.float32)[hs].reshape(1, HG * D).copy(),
        }
        in_maps.append(m)
    return in_maps


def _run(inputs, **kw):
    nc = _get_nc()
    in_maps = _make_in_maps(inputs)
    res = run_bass_kernel_spmd(nc, in_maps, list(range(NCORES)), **kw)
    out = np.zeros((B, S, M), np.float32)
    for core in range(NCORES):
        out[core // HG] += res.results[core]["out_p"]
    out += np.asarray(inputs["b_O"], np.float32)
    return out, res


def kernel(**inputs):
    out, _ = _run(inputs)
    return out


# revision 13
# speedup vs baseline: 1.3951x; 1.2942x over previous
"""Trainium2 Bass/Tile kernel: fused fp8-quantized multi-head causal attention.

Module: q/k/v = fp8(x) @ fp8(W) + b ; scores = (q k^T)/sqrt(64) with causal
mask (-1000 => exp underflows to exactly 0) ; out = softmax(scores) @ v @ W_O + b_O.

Sharding (8 NeuronCores, SPMD, no collectives):
  core c -> batch b = c // 4, head group hg = c % 4 (heads 4*hg .. 4*hg+3).
  Each core returns a partial [S, M] output (its 4 heads' contribution);
  the host sums the 4 partials per batch and adds b_O.

Host-side preprocessing: inputs/W_{Q,K,V} are quantized to fp8-e4m3 on the
host (bit-identical to the reference's jnp e4m3fn cast for |x| <= 240) and
activations are uploaded transposed [M, S] so the contraction dim lands on
SBUF partitions. W_O is uploaded as bf16.

On-chip layout / dataflow per core:
  qT, kT   : [d'=256, S]  (d' = 4 heads x 64), DoubleRow fp8 matmuls,
             bias folded into the DVE eviction (per-partition scalar),
             q scaled by 1/8 (exact, exponent-only) at eviction -> bf16.
  v        : [S, 4x(64+ones-col)] -> bf16 (ones col makes the z^T matmul
             accumulate the softmax denominator in row 64 for free).
  scores^T : psum [sk=128, sq=512] = kT_h(lhsT [64,128]) @ qT_h  (K=64).
             Heads are processed in even/odd pairs whose lhsT/rhs live at
             partition bases 0/64 -> row-group-packed concurrent matmuls.
  pattern  : exp on ScalarE; diagonal tiles get restricted-width exp, a
             gpsimd zero-fill of the fully-masked band and one [128,128]
             triangular bf16 mask multiply on the boundary band.
  z^T+denom: psum [65, sq] += v_h(lhsT [sk,65]) @ pattern.
  normalize: 1/denom via batched reciprocal_approx_accurate (~2 ULP) on the
             head-pair's two denom rows; broadcast across the 64 partitions
             with a K=1 ones matmul; zT_norm = zT * recip -> bf16.
  out      : psum [s=128, m=512] = zt(lhsT [hd=128,s])^T @ W_O, 2 hd chunks,
             evict fp32 -> DRAM partial.
"""

import os
import sys

for _p in ("/opt/trn_rl_repo", os.path.expanduser("~/.axon_site/_ro/trn_rl_repo")):
    if os.path.isdir(_p) and _p not in sys.path:
        sys.path.insert(0, _p)

import ml_dtypes
import numpy as np

import concourse.bass as bass
import concourse.mybir as mybir
import concourse.tile as tile
from concourse import bacc
from concourse.bass_utils import run_bass_kernel_spmd

B, S, M, H, D = 2, 2048, 1024, 16, 64
HG = 4                 # heads per core
NCORES = 8
SQ = 512               # sq chunk width (one fp32 psum bank)
NSQ = S // SQ          # 4
NMC = M // 128         # 8 contraction chunks for projections
NSS = S // 128         # 16 s sub-chunks of 128

F8 = mybir.dt.float8e4
BF = mybir.dt.bfloat16
F32 = mybir.dt.float32
EXP = mybir.ActivationFunctionType.Exp
DR = mybir.MatmulPerfMode.DoubleRow

_f8 = ml_dtypes.float8_e4m3
_bf16 = ml_dtypes.bfloat16


def _build_nc():
    nc = bacc.Bacc(
        "TRN2", target_bir_lowering=False, debug=False, num_devices=NCORES
    )

    xq = nc.declare_dram_parameter("xq_t8", [M, S], F8, isOutput=False)
    xk = nc.declare_dram_parameter("xk_t8", [M, S], F8, isOutput=False)
    xv = nc.declare_dram_parameter("xv_t8", [M, S], F8, isOutput=False)
    wq = nc.declare_dram_parameter("wq8", [M, HG * D], F8, isOutput=False)
    wk = nc.declare_dram_parameter("wk8", [M, HG * D], F8, isOutput=False)
    wv = nc.declare_dram_parameter("wv8", [M, HG * D], F8, isOutput=False)
    wo = nc.declare_dram_parameter("wo_bf", [HG * D, M], BF, isOutput=False)
    bq = nc.declare_dram_parameter("bq", [128, 2], F32, isOutput=False)
    bk = nc.declare_dram_parameter("bk", [128, 2], F32, isOutput=False)
    bv = nc.declare_dram_parameter("bv", [1, HG * D], F32, isOutput=False)
    out_p = nc.declare_dram_parameter("out_p", [S, M], F32, isOutput=True)

    with tile.TileContext(nc) as tc:
        with (
            tc.tile_pool(name="persist", bufs=1) as pers,
            tc.tile_pool(name="work", bufs=6) as work,
            tc.tile_pool(name="ppa", bufs=1, space="PSUM") as ppa,
            tc.tile_pool(name="pps", bufs=4, space="PSUM") as pps,
            tc.tile_pool(name="ppz", bufs=3, space="PSUM") as ppz,
        ):
            # ---- persistent SBUF tensors ----
            xq_sb = pers.tile([128, NMC, S], F8, tag="xq")
            xk_sb = pers.tile([128, NMC, S], F8, tag="xk")
            xv_sb = pers.tile([128, NMC, S], F8, tag="xv")
            wq_sb = pers.tile([128, NMC, HG * D], F8, tag="wq")
            wk_sb = pers.tile([128, NMC, HG * D], F8, tag="wk")
            wv_sb = pers.tile([128, NMC, HG * D], F8, tag="wv")
            wo_sb = pers.tile([128, 2, M], BF, tag="wo")
            bq_sb = pers.tile([128, 2], F32, tag="bq")
            bk_sb = pers.tile([128, 2], F32, tag="bk")
            bv_sb = pers.tile([1, HG * D], F32, tag="bv")
            qt_sb = pers.tile([128, 2, S], BF, tag="qt")
            kt_sb = pers.tile([128, 2, S], BF, tag="kt")
            zt_sb = pers.tile([128, 2, S], BF, tag="zt")
            v_sb = pers.tile([128, NSS, HG, D + 1], BF, tag="v")
            trimask = pers.tile([128, 128], BF, tag="trimask")
            ones = pers.tile([1, SQ], F32, tag="ones")

            # ---- constants ----
            nc.gpsimd.memset(ones[:, :], 1.0)
            nc.gpsimd.memset(v_sb[:, :, :, D : D + 1], 1.0)
            # lower-triangular (inclusive) band mask: keep where row <= col
            nc.gpsimd.memset(trimask[:, :], 1.0)
            nc.gpsimd.affine_select(
                out=trimask[:, :],
                in_=trimask[:, :],
                compare_op=mybir.AluOpType.is_ge,
                fill=0.0,
                base=0,
                pattern=[[1, 128]],
                channel_multiplier=-1,
            )
            # warm the exp table set during the DMA phase
            expwarm = pers.tile([1, 1], F32, tag="expwarm")
            nc.scalar.activation(expwarm[:, :], ones[0:1, 0:1], EXP)

            # ---- input DMAs: weights/biases first, then 2 large DMAs per x ----
            nc.sync.dma_start(
                out=wq_sb[:, :, :], in_=wq[:, :].rearrange("(c p) d -> p c d", p=128)
            )
            nc.sync.dma_start(
                out=wk_sb[:, :, :], in_=wk[:, :].rearrange("(c p) d -> p c d", p=128)
            )
            nc.sync.dma_start(
                out=wv_sb[:, :, :], in_=wv[:, :].rearrange("(c p) d -> p c d", p=128)
            )
            nc.sync.dma_start(out=bq_sb[:, :], in_=bq[:, :])
            nc.sync.dma_start(out=bk_sb[:, :], in_=bk[:, :])
            nc.sync.dma_start(out=bv_sb[:, :], in_=bv[:, :])
            for c in range(2):
                nc.sync.dma_start(out=wo_sb[:, c, :], in_=wo[128 * c : 128 * c + 128, :])
            for x_sb, x_dram in ((xq_sb, xq), (xk_sb, xk), (xv_sb, xv)):
                for g in range(2):  # m-chunks [0:4) then [4:8)
                    nc.sync.dma_start(
                        out=x_sb[:, 4 * g : 4 * g + 4, :],
                        in_=x_dram[512 * g : 512 * g + 512, :].rearrange(
                            "(c p) s -> p c s", p=128
                        ),
                    )

            # ---- phase 1a: qT / kT projections ([d', s], fp8 DoubleRow) ----
            for t in range(NSQ):
                ssl = slice(SQ * t, SQ * t + SQ)
                for half in range(2):
                    dsl = slice(128 * half, 128 * half + 128)
                    for dst_sb, w_sb, x_sb, b_sb, scale in (
                        (qt_sb, wq_sb, xq_sb, bq_sb, 0.125),
                        (kt_sb, wk_sb, xk_sb, bk_sb, None),
                    ):
                        ps = ppa.tile([128, SQ], F32, tag="ppa")
                        for mi in range(0, NMC, 2):
                            nc.tensor.matmul(
                                ps[:, :],
                                lhsT=w_sb[:, mi : mi + 2, dsl],
                                rhs=x_sb[:, mi : mi + 2, ssl],
                                start=(mi == 0),
                                stop=(mi == NMC - 2),
                                perf_mode=DR,
                            )
                        if scale is None:
                            nc.vector.tensor_scalar_add(
                                dst_sb[:, half, ssl], ps[:, :], b_sb[:, half : half + 1]
                            )
                        else:
                            nc.vector.tensor_scalar(
                                out=dst_sb[:, half, ssl],
                                in0=ps[:, :],
                                scalar1=b_sb[:, half : half + 1],
                                scalar2=scale,
                                op0=mybir.AluOpType.add,
                                op1=mybir.AluOpType.mult,
                            )

            # ---- phase 1b: v projection ([s, d'], ones col interleaved) ----
            for ss in range(NSS):
                psl = slice(128 * ss, 128 * ss + 128)
                ps = ppa.tile([128, SQ], F32, tag="ppa")
                for mi in range(0, NMC, 2):
                    nc.tensor.matmul(
                        ps[:, 0 : HG * D],
                        lhsT=xv_sb[:, mi : mi + 2, psl],
                        rhs=wv_sb[:, mi : mi + 2, :],
                        start=(mi == 0),
                        stop=False,
                        perf_mode=DR,
                    )
                nc.tensor.matmul(
                    ps[:, 0 : HG * D],
                    lhsT=ones[0:1, 0:128],
                    rhs=bv_sb[0:1, :],
                    start=False,
                    stop=True,
                )
                nc.vector.tensor_copy(
                    v_sb[:, ss, :, 0:D],
                    ps[:, 0 : HG * D].rearrange("p (g d) -> p g d", g=HG),
                )

            # ---- phase 2: attention, even/odd head pairs (row-group packed) ----
            for jq in range(NSQ):
                qsl = slice(SQ * jq, SQ * jq + SQ)
                nsk = 4 * (jq + 1)
                for c in range(2):  # head pair: heads (2c, 2c+1)
                    ps_z = [
                        ppz.tile([D + 1, SQ], F32, tag="ppz", name=f"psz{jq}_{c}_{u}")
                        for u in range(2)
                    ]
                    p_tiles = {}
                    for si in range(nsk):
                        ksl = slice(128 * si, 128 * si + 128)
                        r = si - 4 * jq  # >=0 on diagonal-band tiles
                        w0 = 128 * r if r > 0 else 0  # fully-masked prefix width
                        for u in range(2):  # head 2c+u at partition base 64*u
                            hsl = slice(64 * u, 64 * u + 64)
                            ps_s = pps.tile([128, SQ], F32, tag="pps")
                            nc.tensor.matmul(
                                ps_s[:, w0:SQ],
                                lhsT=kt_sb[hsl, c, ksl],
                                rhs=qt_sb[hsl, c, SQ * jq + w0 : SQ * jq + SQ],
                                start=True,
                                stop=True,
                            )
                            p_bf = work.tile([128, SQ], BF, tag="p")
                            nc.scalar.activation(
                                p_bf[:, w0:SQ], ps_s[:, w0:SQ], EXP
                            )
                            if r >= 0:
                                nc.vector.tensor_mul(
                                    p_bf[:, w0 : w0 + 128],
                                    p_bf[:, w0 : w0 + 128],
                                    trimask[:, :],
                                )
                            p_tiles[u] = p_bf
                        for u in range(2):
                            h = 2 * c + u
                            nc.tensor.matmul(
                                ps_z[u][:, w0:SQ],
                                lhsT=v_sb[:, si, h, :],
                                rhs=p_tiles[u][:, w0:SQ],
                                start=(si == 0),
                                stop=(si == nsk - 1),
                            )
                    # normalize both heads of the pair
                    recip = work.tile([1, 2 * SQ], F32, tag="recip")
                    scratch = work.tile([1, 2 * SQ], F32, tag="rscr")
                    dn = work.tile([1, 2 * SQ], F32, tag="dn")
                    for u in range(2):
                        nc.vector.tensor_copy(
                            dn[0:1, SQ * u : SQ * u + SQ], ps_z[u][D : D + 1, :]
                        )
                    nc.vector.reciprocal_approx_accurate(
                        out=recip[:, :], in_=dn[:, :], scratch=scratch[:, :]
                    )
                    for u in range(2):
                        ps_b = ppa.tile([128, SQ], F32, tag="ppa")
                        nc.tensor.matmul(
                            ps_b[0:D, :],
                            lhsT=ones[0:1, 0:D],
                            rhs=recip[0:1, SQ * u : SQ * u + SQ],
                            start=True,
                            stop=True,
                        )
                        rb = work.tile([D, SQ], F32, tag="rb")
                        nc.vector.tensor_copy(rb[:, :], ps_b[0:D, :])
                        nc.vector.tensor_mul(
                            zt_sb[64 * u : 64 * u + 64, c, qsl],
                            ps_z[u][0:D, :],
                            rb[:, :],
                        )

            # ---- phase 3: output projection ----
            for ss in range(NSS):
                psl = slice(128 * ss, 128 * ss + 128)
                for n in range(M // SQ):
                    nsl = slice(SQ * n, SQ * n + SQ)
                    ps = ppa.tile([128, SQ], F32, tag="ppa")
                    for c in range(2):
                        nc.tensor.matmul(
                            ps[:, :],
                            lhsT=zt_sb[:, c, psl],
                            rhs=wo_sb[:, c, nsl],
                            start=(c == 0),
                            stop=(c == 1),
                        )
                    o_sb = work.tile([128, SQ], F32, tag="o")
                    nc.vector.tensor_copy(o_sb[:, :], ps[:, :])
                    nc.sync.dma_start(out=out_p[psl, nsl], in_=o_sb[:, :])

    if not nc.is_finalized():
        nc.finalize()
    return nc


_NC = None


def _get_nc():
    global _NC
    if _NC is None:
        _NC = _build_nc()
    return _NC


def _make_in_maps(inputs):
    q8 = lambda a: np.asarray(a, np.float32).astype(_f8)
    xt = {}
    for name, key in (("xq_t8", "query_input"), ("xk_t8", "key_input"),
                      ("xv_t8", "value_input")):
        xt[name] = [np.ascontiguousarray(q8(inputs[key][b]).T) for b in range(B)]

    wq8 = q8(inputs["W_Q"])  # [H, M, D]
    wk8 = q8(inputs["W_K"])
    wv8 = q8(inputs["W_V"])
    wo = np.asarray(inputs["W_O"], np.float32)  # [H, D, M]

    in_maps = []
    for core in range(NCORES):
        b, hg = core // HG, core % HG
        hs = slice(HG * hg, HG * hg + HG)
        m = {
            "xq_t8": xt["xq_t8"][b],
            "xk_t8": xt["xk_t8"][b],
            "xv_t8": xt["xv_t8"][b],
            "wq8": np.ascontiguousarray(wq8[hs].transpose(1, 0, 2).reshape(M, HG * D)),
            "wk8": np.ascontiguousarray(wk8[hs].transpose(1, 0, 2).reshape(M, HG * D)),
            "wv8": np.ascontiguousarray(wv8[hs].transpose(1, 0, 2).reshape(M, HG * D)),
            "wo_bf": np.ascontiguousarray(wo[hs].reshape(HG * D, M).astype(_bf16)),
            "bq": np.ascontiguousarray(np.asarray(inputs["b_Q"], np.float32)[hs].reshape(2, 128).T),
            "bk": np.ascontiguousarray(np.asarray(inputs["b_K"], np.float32)[hs].reshape(2, 128).T),
            "bv": np.asarray(inputs["b_V"], np.float32)[hs].reshape(1, HG * D).copy(),
        }
        in_maps.append(m)
    return in_maps


def _run(inputs, **kw):
    nc = _get_nc()
    in_maps = _make_in_maps(inputs)
    res = run_bass_kernel_spmd(nc, in_maps, list(range(NCORES)), **kw)
    out = np.zeros((B, S, M), np.float32)
    for core in range(NCORES):
        out[core // HG] += res.results[core]["out_p"]
    out += np.asarray(inputs["b_O"], np.float32)
    return out, res


def kernel(**inputs):
    out, _ = _run(inputs)
    return out


# revision 15
# speedup vs baseline: 1.4680x; 1.0522x over previous
"""Trainium2 Bass/Tile kernel: fused fp8-quantized multi-head causal attention.

Module: q/k/v = fp8(x) @ fp8(W) + b ; scores = (q k^T)/sqrt(64) with causal
mask (-1000 => exp underflows to exactly 0) ; out = softmax(scores) @ v @ W_O + b_O.

Sharding (8 NeuronCores, SPMD, no collectives):
  core c -> batch b = c // 4, head group hg = c % 4 (heads 4*hg .. 4*hg+3).
  Each core returns a partial [S, M] output (its 4 heads' contribution);
  the host sums the 4 partials per batch and adds b_O.

Host-side preprocessing: inputs/W_{Q,K,V} are quantized to fp8-e4m3 on the
host (bit-identical to the reference's jnp e4m3fn cast for |x| <= 240) and
activations are uploaded transposed [M, S] so the contraction dim lands on
SBUF partitions. W_O is uploaded as bf16.

On-chip layout / dataflow per core:
  qT, kT   : [d'=256, S]  (d' = 4 heads x 64), DoubleRow fp8 matmuls,
             bias folded into the DVE eviction (per-partition scalar),
             q scaled by 1/8 (exact, exponent-only) at eviction -> bf16.
  v        : [S, 4x(64+ones-col)] -> bf16 (ones col makes the z^T matmul
             accumulate the softmax denominator in row 64 for free).
  scores^T : psum [sk=128, sq=512] = kT_h(lhsT [64,128]) @ qT_h  (K=64).
             Heads are processed in even/odd pairs whose lhsT/rhs live at
             partition bases 0/64 -> row-group-packed concurrent matmuls.
  pattern  : exp on ScalarE; diagonal tiles get restricted-width exp, a
             gpsimd zero-fill of the fully-masked band and one [128,128]
             triangular bf16 mask multiply on the boundary band.
  z^T+denom: psum [65, sq] += v_h(lhsT [sk,65]) @ pattern.
  normalize: 1/denom via batched reciprocal_approx_accurate (~2 ULP) on the
             head-pair's two denom rows; broadcast across the 64 partitions
             with a K=1 ones matmul; zT_norm = zT * recip -> bf16.
  out      : psum [s=128, m=512] = zt(lhsT [hd=128,s])^T @ W_O, 2 hd chunks,
             evict fp32 -> DRAM partial.
"""

import os
import sys

for _p in ("/opt/trn_rl_repo", os.path.expanduser("~/.axon_site/_ro/trn_rl_repo")):
    if os.path.isdir(_p) and _p not in sys.path:
        sys.path.insert(0, _p)

import ml_dtypes
import numpy as np

import concourse.bass as bass
import concourse.mybir as mybir
import concourse.tile as tile
from concourse import bacc
from concourse.bass_utils import run_bass_kernel_spmd

B, S, M, H, D = 2, 2048, 1024, 16, 64
HG = 4                 # heads per core
NCORES = 8
SQ = 512               # sq chunk width (one fp32 psum bank)
NSQ = S // SQ          # 4
NMC = M // 128         # 8 contraction chunks for projections
NSS = S // 128         # 16 s sub-chunks of 128

F8 = mybir.dt.float8e4
BF = mybir.dt.bfloat16
F32 = mybir.dt.float32
EXP = mybir.ActivationFunctionType.Exp
DR = mybir.MatmulPerfMode.DoubleRow

_f8 = ml_dtypes.float8_e4m3
_bf16 = ml_dtypes.bfloat16


def _build_nc():
    nc = bacc.Bacc(
        "TRN2", target_bir_lowering=False, debug=False, num_devices=NCORES
    )

    xq = nc.declare_dram_parameter("xq_t8", [M, S], F8, isOutput=False)
    xk = nc.declare_dram_parameter("xk_t8", [M, S], F8, isOutput=False)
    xv = nc.declare_dram_parameter("xv_t8", [M, S], F8, isOutput=False)
    wq = nc.declare_dram_parameter("wq8", [M, HG * D], F8, isOutput=False)
    wk = nc.declare_dram_parameter("wk8", [M, HG * D], F8, isOutput=False)
    wv = nc.declare_dram_parameter("wv8", [M, HG * D], F8, isOutput=False)
    wo = nc.declare_dram_parameter("wo_bf", [HG * D, M], BF, isOutput=False)
    bq = nc.declare_dram_parameter("bq", [128, 2], F32, isOutput=False)
    bk = nc.declare_dram_parameter("bk", [128, 2], F32, isOutput=False)
    bv = nc.declare_dram_parameter("bv", [1, HG * D], F32, isOutput=False)
    out_p = nc.declare_dram_parameter("out_p", [S, M], F32, isOutput=True)

    with tile.TileContext(nc) as tc:
        with (
            tc.tile_pool(name="persist", bufs=1) as pers,
            tc.tile_pool(name="work", bufs=6) as work,
            tc.tile_pool(name="ppa", bufs=2, space="PSUM") as ppa,
            tc.tile_pool(name="pps", bufs=3, space="PSUM") as pps,
            tc.tile_pool(name="ppz", bufs=3, space="PSUM") as ppz,
        ):
            # ---- persistent SBUF tensors ----
            xq_sb = pers.tile([128, NMC, S], F8, tag="xq")
            xk_sb = pers.tile([128, NMC, S], F8, tag="xk")
            xv_sb = pers.tile([128, NMC, S], F8, tag="xv")
            wq_sb = pers.tile([128, NMC, HG * D], F8, tag="wq")
            wk_sb = pers.tile([128, NMC, HG * D], F8, tag="wk")
            wv_sb = pers.tile([128, NMC, HG * D], F8, tag="wv")
            wo_sb = pers.tile([128, 2, M], BF, tag="wo")
            bq_sb = pers.tile([128, 2], F32, tag="bq")
            bk_sb = pers.tile([128, 2], F32, tag="bk")
            bv_sb = pers.tile([1, HG * D], F32, tag="bv")
            qt_sb = pers.tile([128, 2, S], BF, tag="qt")
            kt_sb = pers.tile([128, 2, S], BF, tag="kt")
            zt_sb = pers.tile([128, 2, S], BF, tag="zt")
            v_sb = pers.tile([128, NSS, HG, D + 1], BF, tag="v")
            trimask = pers.tile([128, 128], BF, tag="trimask")
            ones = pers.tile([1, SQ], F32, tag="ones")

            # ---- constants ----
            nc.gpsimd.memset(ones[:, :], 1.0)
            nc.gpsimd.memset(v_sb[:, :, :, D : D + 1], 1.0)
            # lower-triangular (inclusive) band mask: keep where row <= col
            nc.gpsimd.memset(trimask[:, :], 1.0)
            nc.gpsimd.affine_select(
                out=trimask[:, :],
                in_=trimask[:, :],
                compare_op=mybir.AluOpType.is_ge,
                fill=0.0,
                base=0,
                pattern=[[1, 128]],
                channel_multiplier=-1,
            )
            # warm the exp table set during the DMA phase
            expwarm = pers.tile([1, 1], F32, tag="expwarm")
            nc.scalar.activation(expwarm[:, :], ones[0:1, 0:1], EXP)

            # ---- input DMAs: weights/biases first, then 2 large DMAs per x ----
            nc.sync.dma_start(
                out=wq_sb[:, :, :], in_=wq[:, :].rearrange("(c p) d -> p c d", p=128)
            )
            nc.sync.dma_start(
                out=wk_sb[:, :, :], in_=wk[:, :].rearrange("(c p) d -> p c d", p=128)
            )
            nc.sync.dma_start(
                out=wv_sb[:, :, :], in_=wv[:, :].rearrange("(c p) d -> p c d", p=128)
            )
            nc.sync.dma_start(out=bq_sb[:, :], in_=bq[:, :])
            nc.sync.dma_start(out=bk_sb[:, :], in_=bk[:, :])
            nc.sync.dma_start(out=bv_sb[:, :], in_=bv[:, :])
            for c in range(2):
                nc.sync.dma_start(out=wo_sb[:, c, :], in_=wo[128 * c : 128 * c + 128, :])
            for x_sb, x_dram in ((xq_sb, xq), (xk_sb, xk), (xv_sb, xv)):
                for g in range(2):  # m-chunks [0:4) then [4:8)
                    nc.sync.dma_start(
                        out=x_sb[:, 4 * g : 4 * g + 4, :],
                        in_=x_dram[512 * g : 512 * g + 512, :].rearrange(
                            "(c p) s -> p c s", p=128
                        ),
                    )

            # ---- phase 1a: qT / kT projections ([d', s], fp8 DoubleRow) ----
            for t in range(NSQ):
                ssl = slice(SQ * t, SQ * t + SQ)
                for half in range(2):
                    dsl = slice(128 * half, 128 * half + 128)
                    for dst_sb, w_sb, x_sb, b_sb, scale in (
                        (qt_sb, wq_sb, xq_sb, bq_sb, 0.125),
                        (kt_sb, wk_sb, xk_sb, bk_sb, 1.0),
                    ):
                        ps = ppa.tile([128, SQ], F32, tag="ppa")
                        for mi in range(0, NMC, 2):
                            nc.tensor.matmul(
                                ps[:, :],
                                lhsT=w_sb[:, mi : mi + 2, dsl],
                                rhs=x_sb[:, mi : mi + 2, ssl],
                                start=(mi == 0),
                                stop=(mi == NMC - 2),
                                perf_mode=DR,
                            )
                        # (x@W)*scale + b_pre on ScalarE (b pre-scaled on host)
                        nc.scalar.activation(
                            dst_sb[:, half, ssl],
                            ps[:, :],
                            mybir.ActivationFunctionType.Identity,
                            bias=b_sb[:, half : half + 1],
                            scale=scale,
                        )

            # ---- phase 1b: v projection ([s, d'], ones col interleaved) ----
            for ss in range(NSS):
                psl = slice(128 * ss, 128 * ss + 128)
                ps = ppa.tile([128, SQ], F32, tag="ppa")
                for mi in range(0, NMC, 2):
                    nc.tensor.matmul(
                        ps[:, 0 : HG * D],
                        lhsT=xv_sb[:, mi : mi + 2, psl],
                        rhs=wv_sb[:, mi : mi + 2, :],
                        start=(mi == 0),
                        stop=False,
                        perf_mode=DR,
                    )
                nc.tensor.matmul(
                    ps[:, 0 : HG * D],
                    lhsT=ones[0:1, 0:128],
                    rhs=bv_sb[0:1, :],
                    start=False,
                    stop=True,
                )
                nc.scalar.copy(
                    v_sb[:, ss, :, 0:D],
                    ps[:, 0 : HG * D].rearrange("p (g d) -> p g d", g=HG),
                )

            # ---- phase 2: attention, even/odd head pairs (row-group packed) ----
            for jq in range(NSQ):
                qsl = slice(SQ * jq, SQ * jq + SQ)
                nsk = 4 * (jq + 1)
                for c in range(2):  # head pair: heads (2c, 2c+1)
                    ps_z = [
                        ppz.tile([D + 1, SQ], F32, tag="ppz", name=f"psz{jq}_{c}_{u}")
                        for u in range(2)
                    ]
                    p_tiles = {}
                    for si in range(nsk):
                        ksl = slice(128 * si, 128 * si + 128)
                        r = si - 4 * jq  # >=0 on diagonal-band tiles
                        w0 = 128 * r if r > 0 else 0  # fully-masked prefix width
                        for u in range(2):  # head 2c+u at partition base 64*u
                            hsl = slice(64 * u, 64 * u + 64)
                            ps_s = pps.tile([128, SQ], F32, tag="pps")
                            nc.tensor.matmul(
                                ps_s[:, w0:SQ],
                                lhsT=kt_sb[hsl, c, ksl],
                                rhs=qt_sb[hsl, c, SQ * jq + w0 : SQ * jq + SQ],
                                start=True,
                                stop=True,
                            )
                            p_bf = work.tile([128, SQ], BF, tag="p")
                            nc.scalar.activation(
                                p_bf[:, w0:SQ], ps_s[:, w0:SQ], EXP
                            )
                            if r >= 0:
                                nc.vector.tensor_mul(
                                    p_bf[:, w0 : w0 + 128],
                                    p_bf[:, w0 : w0 + 128],
                                    trimask[:, :],
                                )
                            p_tiles[u] = p_bf
                        for u in range(2):
                            h = 2 * c + u
                            nc.tensor.matmul(
                                ps_z[u][:, w0:SQ],
                                lhsT=v_sb[:, si, h, :],
                                rhs=p_tiles[u][:, w0:SQ],
                                start=(si == 0),
                                stop=(si == nsk - 1),
                            )
                    # normalize both heads of the pair
                    recip = work.tile([1, 2 * SQ], F32, tag="recip")
                    scratch = work.tile([1, 2 * SQ], F32, tag="rscr")
                    dn = work.tile([1, 2 * SQ], F32, tag="dn")
                    for u in range(2):
                        nc.vector.tensor_copy(
                            dn[0:1, SQ * u : SQ * u + SQ], ps_z[u][D : D + 1, :]
                        )
                    nc.vector.reciprocal_approx_accurate(
                        out=recip[:, :], in_=dn[:, :], scratch=scratch[:, :]
                    )
                    for u in range(2):
                        ps_b = pps.tile([128, SQ], F32, tag="pps")
                        nc.tensor.matmul(
                            ps_b[0:D, :],
                            lhsT=ones[0:1, 0:D],
                            rhs=recip[0:1, SQ * u : SQ * u + SQ],
                            start=True,
                            stop=True,
                        )
                        rb = work.tile([D, SQ], F32, tag="rb")
                        nc.vector.tensor_copy(rb[:, :], ps_b[0:D, :])
                        nc.vector.tensor_mul(
                            zt_sb[64 * u : 64 * u + 64, c, qsl],
                            ps_z[u][0:D, :],
                            rb[:, :],
                        )

                # ---- output projection for this jq's s-window ----
                for ss4 in range(4):
                    psl = slice(SQ * jq + 128 * ss4, SQ * jq + 128 * ss4 + 128)
                    for n in range(M // SQ):
                        nsl = slice(SQ * n, SQ * n + SQ)
                        ps_o = ppa.tile([128, SQ], F32, tag="ppa")
                        for c in range(2):
                            nc.tensor.matmul(
                                ps_o[:, :],
                                lhsT=zt_sb[:, c, psl],
                                rhs=wo_sb[:, c, nsl],
                                start=(c == 0),
                                stop=(c == 1),
                            )
                        o_sb = work.tile([128, SQ], F32, tag="o")
                        nc.vector.tensor_copy(o_sb[:, :], ps_o[:, :])
                        nc.sync.dma_start(out=out_p[psl, nsl], in_=o_sb[:, :])

    if not nc.is_finalized():
        nc.finalize()
    return nc


_NC = None


def _get_nc():
    global _NC
    if _NC is None:
        _NC = _build_nc()
    return _NC


def _make_in_maps(inputs):
    q8 = lambda a: np.asarray(a, np.float32).astype(_f8)
    xt = {}
    for name, key in (("xq_t8", "query_input"), ("xk_t8", "key_input"),
                      ("xv_t8", "value_input")):
        xt[name] = [np.ascontiguousarray(q8(inputs[key][b]).T) for b in range(B)]

    wq8 = q8(inputs["W_Q"])  # [H, M, D]
    wk8 = q8(inputs["W_K"])
    wv8 = q8(inputs["W_V"])
    wo = np.asarray(inputs["W_O"], np.float32)  # [H, D, M]

    in_maps = []
    for core in range(NCORES):
        b, hg = core // HG, core % HG
        hs = slice(HG * hg, HG * hg + HG)
        m = {
            "xq_t8": xt["xq_t8"][b],
            "xk_t8": xt["xk_t8"][b],
            "xv_t8": xt["xv_t8"][b],
            "wq8": np.ascontiguousarray(wq8[hs].transpose(1, 0, 2).reshape(M, HG * D)),
            "wk8": np.ascontiguousarray(wk8[hs].transpose(1, 0, 2).reshape(M, HG * D)),
            "wv8": np.ascontiguousarray(wv8[hs].transpose(1, 0, 2).reshape(M, HG * D)),
            "wo_bf": np.ascontiguousarray(wo[hs].reshape(HG * D, M).astype(_bf16)),
            "bq": np.ascontiguousarray(np.asarray(inputs["b_Q"], np.float32)[hs].reshape(2, 128).T / 8.0),
            "bk": np.ascontiguousarray(np.asarray(inputs["b_K"], np.float32)[hs].reshape(2, 128).T),
            "bv": np.asarray(inputs["b_V"], np.float32)[hs].reshape(1, HG * D).copy(),
        }
        in_maps.append(m)
    return in_maps


def _run(inputs, **kw):
    nc = _get_nc()
    in_maps = _make_in_maps(inputs)
    res = run_bass_kernel_spmd(nc, in_maps, list(range(NCORES)), **kw)
    out = np.zeros((B, S, M), np.float32)
    for core in range(NCORES):
        out[core // HG] += res.results[core]["out_p"]
    out += np.asarray(inputs["b_O"], np.float32)
    return out, res


def kernel(**inputs):
    out, _ = _run(inputs)
    return out


# revision 17
# speedup vs baseline: 2.0297x; 1.3826x over previous
"""Trainium2 Bass/Tile kernel: fused fp8-quantized multi-head causal attention.

Module: q/k/v = fp8(x) @ fp8(W) + b ; scores = (q k^T)/sqrt(64) with causal
mask (-1000 => exp underflows to exactly 0) ; out = softmax(scores) @ v @ W_O + b_O.

Sharding (8 NeuronCores, SPMD, no collectives):
  core c -> batch b = c // 4, head group hg = c % 4 (heads 4*hg .. 4*hg+3).
  Each core returns a partial [S, M] output (its 4 heads' contribution);
  the host sums the 4 partials per batch and adds b_O.

Host-side preprocessing: inputs/W_{Q,K,V} are quantized to fp8-e4m3 on the
host (bit-identical to the reference's jnp e4m3fn cast for |x| <= 240),
activations are uploaded transposed [M, S] so the contraction dim lands on
SBUF partitions, and weights are uploaded in partition-major layout so
every DMA moves >= 2 KiB contiguous rows. W_O is uploaded as bf16.

On-chip layout / dataflow per core:
  qT, kT   : [d'=256, S]  (d' = 4 heads x 64), DoubleRow fp8 matmuls,
             (x@W + b) * scale fused into the DVE psum eviction -> bf16
             (q's 1/8 score scale is exact in bf16: exponent-only).
  v        : [S, 4x(64+ones-col)] -> bf16 (ones col makes the z^T matmul
             accumulate the softmax denominator in row 64 for free).
  scores^T : one 2-bank psum [sk=128, 2, sq=512] per sk-chunk holds BOTH
             heads of an even/odd pair (lhsT/rhs at partition bases 0/64 ->
             row-group-packed concurrent matmuls, K=64 each). Diagonal-band
             tiles compute only the causally-live column range.
  pattern  : ONE exp per sk-chunk on ScalarE over both heads (2x1024-wide),
             plus a [128,128] triangular bf16 mask multiply on the band.
  z^T+denom: psum [65, sq] += v_h(lhsT [sk,65]) @ pattern.
  normalize: 1/denom via batched reciprocal_approx_accurate (~2 ULP);
             broadcast across 64 partitions with gpsimd partition_broadcast;
             zT_norm = zT * recip -> bf16 (DVE, fused with the eviction).
  out      : psum [s=128, m=512] = zt(lhsT [hd=128,s])^T @ W_O, 2 hd chunks,
             interleaved into the attention loop per sq window.
"""

import os
import sys

for _p in ("/opt/trn_rl_repo", os.path.expanduser("~/.axon_site/_ro/trn_rl_repo")):
    if os.path.isdir(_p) and _p not in sys.path:
        sys.path.insert(0, _p)

import ml_dtypes
import numpy as np

import concourse.bass as bass
import concourse.mybir as mybir
import concourse.tile as tile
from concourse import bacc
from concourse.bass_utils import run_bass_kernel_spmd

B, S, M, H, D = 2, 2048, 1024, 16, 64
HG = 4                 # heads per core
NCORES = 8
SQ = 512               # sq chunk width (one fp32 psum bank)
NSQ = S // SQ          # 4
NMC = M // 128         # 8 contraction chunks for projections
NSS = S // 128         # 16 s sub-chunks of 128

F8 = mybir.dt.float8e4
BF = mybir.dt.bfloat16
F32 = mybir.dt.float32
EXP = mybir.ActivationFunctionType.Exp
DR = mybir.MatmulPerfMode.DoubleRow

_f8 = ml_dtypes.float8_e4m3
_bf16 = ml_dtypes.bfloat16


def _build_nc():
    nc = bacc.Bacc(
        "TRN2", target_bir_lowering=False, debug=False, num_devices=NCORES
    )

    xq = nc.declare_dram_parameter("xq_t8", [M, S], F8, isOutput=False)
    xk = nc.declare_dram_parameter("xk_t8", [M, S], F8, isOutput=False)
    xv = nc.declare_dram_parameter("xv_t8", [M, S], F8, isOutput=False)
    wq = nc.declare_dram_parameter("wq8", [128, NMC * HG * D], F8, isOutput=False)
    wk = nc.declare_dram_parameter("wk8", [128, NMC * HG * D], F8, isOutput=False)
    wv = nc.declare_dram_parameter("wv8", [128, NMC * HG * D], F8, isOutput=False)
    wo = nc.declare_dram_parameter("wo_bf", [HG * D, M], BF, isOutput=False)
    bq = nc.declare_dram_parameter("bq", [128, 2], F32, isOutput=False)
    bk = nc.declare_dram_parameter("bk", [128, 2], F32, isOutput=False)
    bv = nc.declare_dram_parameter("bv", [1, HG * D], F32, isOutput=False)
    out_p = nc.declare_dram_parameter("out_p", [S, M], F32, isOutput=True)

    with tile.TileContext(nc) as tc:
        with (
            tc.tile_pool(name="persist", bufs=1) as pers,
            tc.tile_pool(name="work", bufs=6) as work,
            tc.tile_pool(name="ppa", bufs=1, space="PSUM") as ppa,
            tc.tile_pool(name="pps", bufs=2, space="PSUM") as pps,
            tc.tile_pool(name="ppz", bufs=3, space="PSUM") as ppz,
        ):
            # ---- persistent SBUF tensors ----
            xq_sb = pers.tile([128, NMC, S], F8, tag="xq")
            xk_sb = pers.tile([128, NMC, S], F8, tag="xk")
            xv_sb = pers.tile([128, NMC, S], F8, tag="xv")
            wq_sb = pers.tile([128, NMC, HG * D], F8, tag="wq")
            wk_sb = pers.tile([128, NMC, HG * D], F8, tag="wk")
            wv_sb = pers.tile([128, NMC, HG * D], F8, tag="wv")
            wo_sb = pers.tile([128, 2, M], BF, tag="wo")
            bq_sb = pers.tile([128, 2], F32, tag="bq")
            bk_sb = pers.tile([128, 2], F32, tag="bk")
            bv_sb = pers.tile([1, HG * D], F32, tag="bv")
            qt_sb = pers.tile([128, 2, S], BF, tag="qt")
            kt_sb = pers.tile([128, 2, S], BF, tag="kt")
            zt_sb = pers.tile([128, 2, S], BF, tag="zt")
            v_sb = pers.tile([128, NSS, HG, D + 1], BF, tag="v")
            trimask = pers.tile([128, 128], BF, tag="trimask")
            ones = pers.tile([1, SQ], F32, tag="ones")

            # ---- constants ----
            nc.gpsimd.memset(ones[:, :], 1.0)
            nc.gpsimd.memset(v_sb[:, :, :, D : D + 1], 1.0)
            # lower-triangular (inclusive) band mask: keep where row <= col
            nc.gpsimd.memset(trimask[:, :], 1.0)
            nc.gpsimd.affine_select(
                out=trimask[:, :],
                in_=trimask[:, :],
                compare_op=mybir.AluOpType.is_ge,
                fill=0.0,
                base=0,
                pattern=[[1, 128]],
                channel_multiplier=-1,
            )
            # warm the exp table set during the DMA phase
            expwarm = pers.tile([1, 1], F32, tag="expwarm")
            nc.scalar.activation(expwarm[:, :], ones[0:1, 0:1], EXP)

            # ---- input DMAs: weights/biases first, then 2 large DMAs per x ----
            nc.sync.dma_start(out=wq_sb[:, :, :], in_=wq[:, :])
            nc.sync.dma_start(out=wk_sb[:, :, :], in_=wk[:, :])
            nc.sync.dma_start(out=wv_sb[:, :, :], in_=wv[:, :])
            nc.sync.dma_start(out=bq_sb[:, :], in_=bq[:, :])
            nc.sync.dma_start(out=bk_sb[:, :], in_=bk[:, :])
            nc.sync.dma_start(out=bv_sb[:, :], in_=bv[:, :])
            for c in range(2):
                nc.sync.dma_start(out=wo_sb[:, c, :], in_=wo[128 * c : 128 * c + 128, :])
            for x_sb, x_dram in ((xq_sb, xq), (xk_sb, xk), (xv_sb, xv)):
                for g in range(2):  # m-chunks [0:4) then [4:8)
                    nc.sync.dma_start(
                        out=x_sb[:, 4 * g : 4 * g + 4, :],
                        in_=x_dram[512 * g : 512 * g + 512, :].rearrange(
                            "(c p) s -> p c s", p=128
                        ),
                    )

            # ---- phase 1a: qT / kT projections ([d', s], fp8 DoubleRow) ----
            for t in range(NSQ):
                ssl = slice(SQ * t, SQ * t + SQ)
                for half in range(2):
                    dsl = slice(128 * half, 128 * half + 128)
                    for dst_sb, w_sb, x_sb, b_sb, scale in (
                        (qt_sb, wq_sb, xq_sb, bq_sb, 0.125),
                        (kt_sb, wk_sb, xk_sb, bk_sb, None),
                    ):
                        ps = ppa.tile([128, SQ], F32, tag="ppa")
                        for mi in range(0, NMC, 2):
                            nc.tensor.matmul(
                                ps[:, :],
                                lhsT=w_sb[:, mi : mi + 2, dsl],
                                rhs=x_sb[:, mi : mi + 2, ssl],
                                start=(mi == 0),
                                stop=(mi == NMC - 2),
                                perf_mode=DR,
                            )
                        if scale is None:
                            nc.vector.tensor_scalar_add(
                                dst_sb[:, half, ssl], ps[:, :], b_sb[:, half : half + 1]
                            )
                        else:
                            nc.vector.tensor_scalar(
                                out=dst_sb[:, half, ssl],
                                in0=ps[:, :],
                                scalar1=b_sb[:, half : half + 1],
                                scalar2=scale,
                                op0=mybir.AluOpType.add,
                                op1=mybir.AluOpType.mult,
                            )

            # ---- phase 1b: v projection ([s, d'], ones col interleaved) ----
            for ss in range(NSS):
                psl = slice(128 * ss, 128 * ss + 128)
                ps = ppa.tile([128, SQ], F32, tag="ppa")
                for mi in range(0, NMC, 2):
                    nc.tensor.matmul(
                        ps[:, 0 : HG * D],
                        lhsT=xv_sb[:, mi : mi + 2, psl],
                        rhs=wv_sb[:, mi : mi + 2, :],
                        start=(mi == 0),
                        stop=False,
                        perf_mode=DR,
                    )
                nc.tensor.matmul(
                    ps[:, 0 : HG * D],
                    lhsT=ones[0:1, 0:128],
                    rhs=bv_sb[0:1, :],
                    start=False,
                    stop=True,
                )
                nc.vector.tensor_copy(
                    v_sb[:, ss, :, 0:D],
                    ps[:, 0 : HG * D].rearrange("p (g d) -> p g d", g=HG),
                )

            # ---- phase 2: attention, even/odd head pairs (row-group packed) ----
            for jq in range(NSQ):
                qsl = slice(SQ * jq, SQ * jq + SQ)
                nsk = 4 * (jq + 1)
                for c in range(2):  # head pair: heads (2c, 2c+1)
                    ps_z = [
                        ppz.tile([D + 1, SQ], F32, tag="ppz", name=f"psz{jq}_{c}_{u}")
                        for u in range(2)
                    ]
                    for si in range(nsk):
                        ksl = slice(128 * si, 128 * si + 128)
                        r = si - 4 * jq  # >=0 on diagonal-band tiles
                        w0 = 128 * r if r > 0 else 0  # fully-masked prefix
                        # both heads' scores into one 2-bank psum tile
                        ps2 = pps.tile([128, 2, SQ], F32, tag="pps")
                        for u in range(2):
                            hsl = slice(64 * u, 64 * u + 64)
                            nc.tensor.matmul(
                                ps2[:, u, w0:SQ],
                                lhsT=kt_sb[hsl, c, ksl],
                                rhs=qt_sb[hsl, c, SQ * jq + w0 : SQ * jq + SQ],
                                start=True,
                                stop=True,
                            )
                        p_bf = work.tile([128, 2, SQ], BF, tag="p")
                        nc.scalar.activation(
                            p_bf[:, :, w0:SQ], ps2[:, :, w0:SQ], EXP
                        )
                        if r >= 0:
                            for u in range(2):
                                nc.vector.tensor_mul(
                                    p_bf[:, u, w0 : w0 + 128],
                                    p_bf[:, u, w0 : w0 + 128],
                                    trimask[:, :],
                                )
                        for u in range(2):
                            h = 2 * c + u
                            nc.tensor.matmul(
                                ps_z[u][:, w0:SQ],
                                lhsT=v_sb[:, si, h, :],
                                rhs=p_bf[:, u, w0:SQ],
                                start=(si == 0),
                                stop=(si == nsk - 1),
                            )
                    # normalize both heads of the pair
                    recip = work.tile([1, 2 * SQ], F32, tag="recip")
                    scratch = work.tile([1, 2 * SQ], F32, tag="rscr")
                    dn = work.tile([1, 2 * SQ], F32, tag="dn")
                    for u in range(2):
                        nc.vector.tensor_copy(
                            dn[0:1, SQ * u : SQ * u + SQ], ps_z[u][D : D + 1, :]
                        )
                    nc.vector.reciprocal_approx_accurate(
                        out=recip[:, :], in_=dn[:, :], scratch=scratch[:, :]
                    )
                    for u in range(2):
                        rb = work.tile([D, SQ], F32, tag="rb")
                        nc.gpsimd.partition_broadcast(
                            rb[:, :], recip[0:1, SQ * u : SQ * u + SQ]
                        )
                        nc.vector.tensor_mul(
                            zt_sb[64 * u : 64 * u + 64, c, qsl],
                            ps_z[u][0:D, :],
                            rb[:, :],
                        )
                # ---- output projection for this jq's s-window ----
                for ss4 in range(4):
                    psl = slice(SQ * jq + 128 * ss4, SQ * jq + 128 * ss4 + 128)
                    for n in range(M // SQ):
                        nsl = slice(SQ * n, SQ * n + SQ)
                        ps_o = ppa.tile([128, SQ], F32, tag="ppa")
                        for c in range(2):
                            nc.tensor.matmul(
                                ps_o[:, :],
                                lhsT=zt_sb[:, c, psl],
                                rhs=wo_sb[:, c, nsl],
                                start=(c == 0),
                                stop=(c == 1),
                            )
                        o_sb = work.tile([128, SQ], F32, tag="o")
                        nc.vector.tensor_copy(o_sb[:, :], ps_o[:, :])
                        nc.sync.dma_start(out=out_p[psl, nsl], in_=o_sb[:, :])

    if not nc.is_finalized():
        nc.finalize()
    return nc


_NC = None


def _get_nc():
    global _NC
    if _NC is None:
        _NC = _build_nc()
    return _NC


def _wpack(w):
    """[M, HG*D] -> partition-major [128, NMC*HG*D] (2 KiB contiguous rows)."""
    return np.ascontiguousarray(
        w.reshape(NMC, 128, HG * D).transpose(1, 0, 2).reshape(128, NMC * HG * D)
    )


def _make_in_maps(inputs):
    q8 = lambda a: np.asarray(a, np.float32).astype(_f8)
    xt = {}
    for name, key in (("xq_t8", "query_input"), ("xk_t8", "key_input"),
                      ("xv_t8", "value_input")):
        xt[name] = [np.ascontiguousarray(q8(inputs[key][b]).T) for b in range(B)]

    wq8 = q8(inputs["W_Q"])  # [H, M, D]
    wk8 = q8(inputs["W_K"])
    wv8 = q8(inputs["W_V"])
    wo = np.asarray(inputs["W_O"], np.float32)  # [H, D, M]

    in_maps = []
    for core in range(NCORES):
        b, hg = core // HG, core % HG
        hs = slice(HG * hg, HG * hg + HG)
        m = {
            "xq_t8": xt["xq_t8"][b],
            "xk_t8": xt["xk_t8"][b],
            "xv_t8": xt["xv_t8"][b],
            "wq8": _wpack(wq8[hs].transpose(1, 0, 2).reshape(M, HG * D)),
            "wk8": _wpack(wk8[hs].transpose(1, 0, 2).reshape(M, HG * D)),
            "wv8": _wpack(wv8[hs].transpose(1, 0, 2).reshape(M, HG * D)),
            "wo_bf": np.ascontiguousarray(wo[hs].reshape(HG * D, M).astype(_bf16)),
            "bq": np.ascontiguousarray(
                np.asarray(inputs["b_Q"], np.float32)[hs].reshape(2, 128).T
            ),
            "bk": np.ascontiguousarray(
                np.asarray(inputs["b_K"], np.float32)[hs].reshape(2, 128).T
            ),
            "bv": np.asarray(inputs["b_V"], np.float32)[hs].reshape(1, HG * D).copy(),
        }
        in_maps.append(m)
    return in_maps


def _run(inputs, **kw):
    nc = _get_nc()
    in_maps = _make_in_maps(inputs)
    res = run_bass_kernel_spmd(nc, in_maps, list(range(NCORES)), **kw)
    out = np.zeros((B, S, M), np.float32)
    for core in range(NCORES):
        out[core // HG] += res.results[core]["out_p"]
    out += np.asarray(inputs["b_O"], np.float32)
    return out, res


def kernel(**inputs):
    out, _ = _run(inputs)
    return out


# revision 19
# speedup vs baseline: 2.1473x; 1.0580x over previous
"""Trainium2 Bass/Tile kernel: fused fp8-quantized multi-head causal attention.

Module: q/k/v = fp8(x) @ fp8(W) + b ; scores = (q k^T)/sqrt(64) with causal
mask (-1000 => exp underflows to exactly 0) ; out = softmax(scores) @ v @ W_O + b_O.

Sharding (8 NeuronCores, SPMD, no collectives):
  core c -> batch b = c // 4, head group hg = c % 4 (heads 4*hg .. 4*hg+3).
  Each core returns a partial [S, M] output (its 4 heads' contribution);
  the host sums the 4 partials per batch and adds b_O.

Host-side preprocessing: inputs/W_{Q,K,V} are quantized to fp8-e4m3 on the
host (bit-identical to the reference's jnp e4m3fn cast for |x| <= 240),
activations are uploaded transposed [M, S] so the contraction dim lands on
SBUF partitions, and weights are uploaded in partition-major layout so
every DMA moves >= 2 KiB contiguous rows. W_O is uploaded as bf16.

On-chip layout / dataflow per core:
  qT, kT   : [d'=256, S]  (d' = 4 heads x 64), DoubleRow fp8 matmuls,
             (x@W + b) * scale fused into the DVE psum eviction -> bf16
             (q's 1/8 score scale is exact in bf16: exponent-only).
  v        : [S, 4x(64+ones-col)] -> bf16 (ones col makes the z^T matmul
             accumulate the softmax denominator in row 64 for free).
  scores^T : one 2-bank psum [sk=128, 2, sq=512] per sk-chunk holds BOTH
             heads of an even/odd pair (lhsT/rhs at partition bases 0/64 ->
             row-group-packed concurrent matmuls, K=64 each). Diagonal-band
             tiles compute only the causally-live column range.
  pattern  : ONE exp per sk-chunk on ScalarE over both heads (2x1024-wide),
             plus a [128,128] triangular bf16 mask multiply on the band.
  z^T+denom: psum [65, sq] += v_h(lhsT [sk,65]) @ pattern.
  normalize: 1/denom via batched reciprocal_approx_accurate (~2 ULP);
             broadcast across 64 partitions with gpsimd partition_broadcast;
             zT_norm = zT * recip -> bf16 (DVE, fused with the eviction).
  out      : psum [s=128, m=512] = zt(lhsT [hd=128,s])^T @ W_O, 2 hd chunks,
             interleaved into the attention loop per sq window.
"""

import os
import sys

for _p in ("/opt/trn_rl_repo", os.path.expanduser("~/.axon_site/_ro/trn_rl_repo")):
    if os.path.isdir(_p) and _p not in sys.path:
        sys.path.insert(0, _p)

import ml_dtypes
import numpy as np

import concourse.bass as bass
import concourse.mybir as mybir
import concourse.tile as tile
from concourse import bacc
from concourse.bass_utils import run_bass_kernel_spmd

B, S, M, H, D = 2, 2048, 1024, 16, 64
HG = 4                 # heads per core
NCORES = 8
SQ = 512               # sq chunk width (one fp32 psum bank)
NSQ = S // SQ          # 4
NMC = M // 128         # 8 contraction chunks for projections
NSS = S // 128         # 16 s sub-chunks of 128

F8 = mybir.dt.float8e4
BF = mybir.dt.bfloat16
F32 = mybir.dt.float32
EXP = mybir.ActivationFunctionType.Exp
DR = mybir.MatmulPerfMode.DoubleRow

_f8 = ml_dtypes.float8_e4m3
_bf16 = ml_dtypes.bfloat16


def _build_nc():
    nc = bacc.Bacc(
        "TRN2", target_bir_lowering=False, debug=False, num_devices=NCORES
    )

    xq = nc.declare_dram_parameter("xq_t8", [M, S], F8, isOutput=False)
    xk = nc.declare_dram_parameter("xk_t8", [M, S], F8, isOutput=False)
    xv = nc.declare_dram_parameter("xv_t8", [M, S], F8, isOutput=False)
    wq = nc.declare_dram_parameter("wq8", [128, NMC * HG * D], F8, isOutput=False)
    wk = nc.declare_dram_parameter("wk8", [128, NMC * HG * D], F8, isOutput=False)
    wv = nc.declare_dram_parameter("wv8", [128, NMC * HG * D], F8, isOutput=False)
    wo = nc.declare_dram_parameter("wo_bf", [HG * D, M], BF, isOutput=False)
    bq = nc.declare_dram_parameter("bq", [128, 2], F32, isOutput=False)
    bk = nc.declare_dram_parameter("bk", [128, 2], F32, isOutput=False)
    bv = nc.declare_dram_parameter("bv", [1, HG * D], F32, isOutput=False)
    out_p = nc.declare_dram_parameter("out_p", [S, M], F32, isOutput=True)

    with tile.TileContext(nc) as tc:
        with (
            tc.tile_pool(name="persist", bufs=1) as pers,
            tc.tile_pool(name="work", bufs=6) as work,
            tc.tile_pool(name="ppa", bufs=1, space="PSUM") as ppa,
            tc.tile_pool(name="pps", bufs=2, space="PSUM") as pps,
            tc.tile_pool(name="ppz", bufs=3, space="PSUM") as ppz,
        ):
            # ---- persistent SBUF tensors ----
            xq_sb = pers.tile([128, NMC, S], F8, tag="xq")
            xk_sb = pers.tile([128, NMC, S], F8, tag="xk")
            xv_sb = pers.tile([128, NMC, S], F8, tag="xv")
            wq_sb = pers.tile([128, NMC, HG * D], F8, tag="wq")
            wk_sb = pers.tile([128, NMC, HG * D], F8, tag="wk")
            wv_sb = pers.tile([128, NMC, HG * D], F8, tag="wv")
            wo_sb = pers.tile([128, 2, M], BF, tag="wo")
            bq_sb = pers.tile([128, 2], F32, tag="bq")
            bk_sb = pers.tile([128, 2], F32, tag="bk")
            bv_sb = pers.tile([1, HG * D], F32, tag="bv")
            qt_sb = pers.tile([128, 2, S], BF, tag="qt")
            kt_sb = pers.tile([128, 2, S], BF, tag="kt")
            zt_sb = pers.tile([128, 2, S], BF, tag="zt")
            v_sb = pers.tile([128, NSS, HG, D + 1], BF, tag="v")
            trimask = pers.tile([128, 128], BF, tag="trimask")
            ones = pers.tile([1, SQ], F32, tag="ones")

            # ---- constants ----
            nc.gpsimd.memset(ones[:, :], 1.0)
            nc.gpsimd.memset(v_sb[:, :, :, D : D + 1], 1.0)
            # lower-triangular (inclusive) band mask: keep where row <= col
            nc.gpsimd.memset(trimask[:, :], 1.0)
            nc.gpsimd.affine_select(
                out=trimask[:, :],
                in_=trimask[:, :],
                compare_op=mybir.AluOpType.is_ge,
                fill=0.0,
                base=0,
                pattern=[[1, 128]],
                channel_multiplier=-1,
            )
            # warm the exp table set during the DMA phase
            expwarm = pers.tile([1, 1], F32, tag="expwarm")
            nc.scalar.activation(expwarm[:, :], ones[0:1, 0:1], EXP)

            # ---- input DMAs: weights/biases first, then 2 large DMAs per x ----
            nc.sync.dma_start(out=wq_sb[:, :, :], in_=wq[:, :])
            nc.sync.dma_start(out=wk_sb[:, :, :], in_=wk[:, :])
            nc.sync.dma_start(out=wv_sb[:, :, :], in_=wv[:, :])
            nc.sync.dma_start(out=bq_sb[:, :], in_=bq[:, :])
            nc.sync.dma_start(out=bk_sb[:, :], in_=bk[:, :])
            nc.sync.dma_start(out=bv_sb[:, :], in_=bv[:, :])
            for c in range(2):
                nc.sync.dma_start(out=wo_sb[:, c, :], in_=wo[128 * c : 128 * c + 128, :])
            for x_sb, x_dram in ((xq_sb, xq), (xk_sb, xk), (xv_sb, xv)):
                for g in range(2):  # m-chunks [0:4) then [4:8)
                    nc.sync.dma_start(
                        out=x_sb[:, 4 * g : 4 * g + 4, :],
                        in_=x_dram[512 * g : 512 * g + 512, :].rearrange(
                            "(c p) s -> p c s", p=128
                        ),
                    )

            # ---- phase 1a: qT / kT projections ([d', s], fp8 DoubleRow) ----
            for t in range(NSQ):
                ssl = slice(SQ * t, SQ * t + SQ)
                for half in range(2):
                    dsl = slice(128 * half, 128 * half + 128)
                    for dst_sb, w_sb, x_sb, b_sb, scale in (
                        (qt_sb, wq_sb, xq_sb, bq_sb, 0.125),
                        (kt_sb, wk_sb, xk_sb, bk_sb, None),
                    ):
                        ps2p = pps.tile([128, 2, SQ], F32, tag="pps")
                        ps = ps2p[:, 0, :]
                        for mi in range(0, NMC, 2):
                            nc.tensor.matmul(
                                ps[:, :],
                                lhsT=w_sb[:, mi : mi + 2, dsl],
                                rhs=x_sb[:, mi : mi + 2, ssl],
                                start=(mi == 0),
                                stop=(mi == NMC - 2),
                                perf_mode=DR,
                            )
                        if scale is None:
                            nc.vector.tensor_scalar_add(
                                dst_sb[:, half, ssl], ps[:, :], b_sb[:, half : half + 1]
                            )
                        else:
                            nc.vector.tensor_scalar(
                                out=dst_sb[:, half, ssl],
                                in0=ps[:, :],
                                scalar1=b_sb[:, half : half + 1],
                                scalar2=scale,
                                op0=mybir.AluOpType.add,
                                op1=mybir.AluOpType.mult,
                            )

            # ---- phase 1b: v projection ([s, d'], ones col interleaved) ----
            for ss in range(NSS):
                psl = slice(128 * ss, 128 * ss + 128)
                ps2p = pps.tile([128, 2, SQ], F32, tag="pps")
                ps = ps2p[:, 0, :]
                for mi in range(0, NMC, 2):
                    nc.tensor.matmul(
                        ps[:, 0 : HG * D],
                        lhsT=xv_sb[:, mi : mi + 2, psl],
                        rhs=wv_sb[:, mi : mi + 2, :],
                        start=(mi == 0),
                        stop=False,
                        perf_mode=DR,
                    )
                nc.tensor.matmul(
                    ps[:, 0 : HG * D],
                    lhsT=ones[0:1, 0:128],
                    rhs=bv_sb[0:1, :],
                    start=False,
                    stop=True,
                )
                nc.vector.tensor_copy(
                    v_sb[:, ss, :, 0:D],
                    ps[:, 0 : HG * D].rearrange("p (g d) -> p g d", g=HG),
                )

            # ---- phase 2: attention, even/odd head pairs (row-group packed) ----
            for jq in range(NSQ):
                qsl = slice(SQ * jq, SQ * jq + SQ)
                nsk = 4 * (jq + 1)
                for c in range(2):  # head pair: heads (2c, 2c+1)
                    ps_z = [
                        ppz.tile([D + 1, SQ], F32, tag="ppz", name=f"psz{jq}_{c}_{u}")
                        for u in range(2)
                    ]
                    for si in range(nsk):
                        ksl = slice(128 * si, 128 * si + 128)
                        r = si - 4 * jq  # >=0 on diagonal-band tiles
                        w0 = 128 * r if r > 0 else 0  # fully-masked prefix
                        # both heads' scores into one 2-bank psum tile
                        ps2 = pps.tile([128, 2, SQ], F32, tag="pps")
                        for u in range(2):
                            hsl = slice(64 * u, 64 * u + 64)
                            nc.tensor.matmul(
                                ps2[:, u, w0:SQ],
                                lhsT=kt_sb[hsl, c, ksl],
                                rhs=qt_sb[hsl, c, SQ * jq + w0 : SQ * jq + SQ],
                                start=True,
                                stop=True,
                            )
                        p_bf = work.tile([128, 2, SQ], BF, tag="p")
                        nc.scalar.activation(
                            p_bf[:, :, w0:SQ], ps2[:, :, w0:SQ], EXP
                        )
                        if r >= 0:
                            for u in range(2):
                                nc.vector.tensor_mul(
                                    p_bf[:, u, w0 : w0 + 128],
                                    p_bf[:, u, w0 : w0 + 128],
                                    trimask[:, :],
                                )
                        for u in range(2):
                            h = 2 * c + u
                            nc.tensor.matmul(
                                ps_z[u][:, w0:SQ],
                                lhsT=v_sb[:, si, h, :],
                                rhs=p_bf[:, u, w0:SQ],
                                start=(si == 0),
                                stop=(si == nsk - 1),
                            )
                    # normalize both heads of the pair (~18-bit reciprocal is
                    # far below the bf16 pattern noise floor)
                    for u in range(2):
                        dn = work.tile([1, SQ], F32, tag="dn")
                        nc.vector.tensor_copy(dn[:, :], ps_z[u][D : D + 1, :])
                        recip = work.tile([1, SQ], F32, tag="recip")
                        nc.vector.reciprocal_approx_fast(
                            out=recip[:, :], in_=dn[:, :]
                        )
                        rb = work.tile([D, SQ], F32, tag="rb")
                        nc.gpsimd.partition_broadcast(rb[:, :], recip[0:1, :])
                        nc.vector.tensor_mul(
                            zt_sb[64 * u : 64 * u + 64, c, qsl],
                            ps_z[u][0:D, :],
                            rb[:, :],
                        )
                # ---- output projection for this jq's s-window ----
                for ss4 in range(4):
                    psl = slice(SQ * jq + 128 * ss4, SQ * jq + 128 * ss4 + 128)
                    for n in range(M // SQ):
                        nsl = slice(SQ * n, SQ * n + SQ)
                        ps_o = ppa.tile([128, SQ], F32, tag="ppa")
                        for c in range(2):
                            nc.tensor.matmul(
                                ps_o[:, :],
                                lhsT=zt_sb[:, c, psl],
                                rhs=wo_sb[:, c, nsl],
                                start=(c == 0),
                                stop=(c == 1),
                            )
                        o_sb = work.tile([128, SQ], F32, tag="o")
                        nc.vector.tensor_copy(o_sb[:, :], ps_o[:, :])
                        nc.sync.dma_start(out=out_p[psl, nsl], in_=o_sb[:, :])

    if not nc.is_finalized():
        nc.finalize()
    return nc


_NC = None


def _get_nc():
    global _NC
    if _NC is None:
        _NC = _build_nc()
    return _NC


def _wpack(w):
    """[M, HG*D] -> partition-major [128, NMC*HG*D] (2 KiB contiguous rows)."""
    return np.ascontiguousarray(
        w.reshape(NMC, 128, HG * D).transpose(1, 0, 2).reshape(128, NMC * HG * D)
    )


def _make_in_maps(inputs):
    q8 = lambda a: np.asarray(a, np.float32).astype(_f8)
    xt = {}
    for name, key in (("xq_t8", "query_input"), ("xk_t8", "key_input"),
                      ("xv_t8", "value_input")):
        xt[name] = [np.ascontiguousarray(q8(inputs[key][b]).T) for b in range(B)]

    wq8 = q8(inputs["W_Q"])  # [H, M, D]
    wk8 = q8(inputs["W_K"])
    wv8 = q8(inputs["W_V"])
    wo = np.asarray(inputs["W_O"], np.float32)  # [H, D, M]

    in_maps = []
    for core in range(NCORES):
        b, hg = core // HG, core % HG
        hs = slice(HG * hg, HG * hg + HG)
        m = {
            "xq_t8": xt["xq_t8"][b],
            "xk_t8": xt["xk_t8"][b],
            "xv_t8": xt["xv_t8"][b],
            "wq8": _wpack(wq8[hs].transpose(1, 0, 2).reshape(M, HG * D)),
            "wk8": _wpack(wk8[hs].transpose(1, 0, 2).reshape(M, HG * D)),
            "wv8": _wpack(wv8[hs].transpose(1, 0, 2).reshape(M, HG * D)),
            "wo_bf": np.ascontiguousarray(wo[hs].reshape(HG * D, M).astype(_bf16)),
            "bq": np.ascontiguousarray(
                np.asarray(inputs["b_Q"], np.float32)[hs].reshape(2, 128).T
            ),
            "bk": np.ascontiguousarray(
                np.asarray(inputs["b_K"], np.float32)[hs].reshape(2, 128).T
            ),
            "bv": np.asarray(inputs["b_V"], np.float32)[hs].reshape(1, HG * D).copy(),
        }
        in_maps.append(m)
    return in_maps


def _run(inputs, **kw):
    nc = _get_nc()
    in_maps = _make_in_maps(inputs)
    res = run_bass_kernel_spmd(nc, in_maps, list(range(NCORES)), **kw)
    out = np.zeros((B, S, M), np.float32)
    for core in range(NCORES):
        out[core // HG] += res.results[core]["out_p"]
    out += np.asarray(inputs["b_O"], np.float32)
    return out, res


def kernel(**inputs):
    out, _ = _run(inputs)
    return out


# revision 20
# speedup vs baseline: 2.2519x; 1.0487x over previous
"""Trainium2 Bass/Tile kernel: fused fp8-quantized multi-head causal attention.

Module: q/k/v = fp8(x) @ fp8(W) + b ; scores = (q k^T)/sqrt(64) with causal
mask (-1000 => exp underflows to exactly 0) ; out = softmax(scores) @ v @ W_O + b_O.

Sharding (8 NeuronCores, SPMD, no collectives):
  core c -> batch b = c // 4, head group hg = c % 4 (heads 4*hg .. 4*hg+3).
  Each core returns a partial [S, M] output (its 4 heads' contribution);
  the host sums the 4 partials per batch and adds b_O.

Host-side preprocessing: inputs/W_{Q,K,V} are quantized to fp8-e4m3 on the
host (bit-identical to the reference's jnp e4m3fn cast for |x| <= 240),
activations are uploaded transposed [M, S] so the contraction dim lands on
SBUF partitions, and weights are uploaded in partition-major layout so
every DMA moves >= 2 KiB contiguous rows. W_O is uploaded as bf16.

On-chip layout / dataflow per core:
  qT, kT   : [d'=256, S]  (d' = 4 heads x 64), DoubleRow fp8 matmuls,
             (x@W + b) * scale fused into the DVE psum eviction -> bf16
             (q's 1/8 score scale is exact in bf16: exponent-only).
  v        : [S, 4x(64+ones-col)] -> bf16 (ones col makes the z^T matmul
             accumulate the softmax denominator in row 64 for free).
  scores^T : one 2-bank psum [sk=128, 2, sq=512] per sk-chunk holds BOTH
             heads of an even/odd pair (lhsT/rhs at partition bases 0/64 ->
             row-group-packed concurrent matmuls, K=64 each). Diagonal-band
             tiles compute only the causally-live column range.
  pattern  : ONE exp per sk-chunk on ScalarE over both heads (2x1024-wide),
             plus a [128,128] triangular bf16 mask multiply on the band.
  z^T+denom: psum [65, sq] += v_h(lhsT [sk,65]) @ pattern.
  normalize: 1/denom via batched reciprocal_approx_accurate (~2 ULP);
             broadcast across 64 partitions with gpsimd partition_broadcast;
             zT_norm = zT * recip -> bf16 (DVE, fused with the eviction).
  out      : psum [s=128, m=512] = zt(lhsT [hd=128,s])^T @ W_O, 2 hd chunks,
             interleaved into the attention loop per sq window.
"""

import os
import sys

for _p in ("/opt/trn_rl_repo", os.path.expanduser("~/.axon_site/_ro/trn_rl_repo")):
    if os.path.isdir(_p) and _p not in sys.path:
        sys.path.insert(0, _p)

import ml_dtypes
import numpy as np

import concourse.bass as bass
import concourse.mybir as mybir
import concourse.tile as tile
from concourse import bacc
from concourse.bass_utils import run_bass_kernel_spmd

B, S, M, H, D = 2, 2048, 1024, 16, 64
HG = 4                 # heads per core
NCORES = 8
SQ = 512               # sq chunk width (one fp32 psum bank)
NSQ = S // SQ          # 4
NMC = M // 128         # 8 contraction chunks for projections
NSS = S // 128         # 16 s sub-chunks of 128

F8 = mybir.dt.float8e4
BF = mybir.dt.bfloat16
F32 = mybir.dt.float32
EXP = mybir.ActivationFunctionType.Exp
DR = mybir.MatmulPerfMode.DoubleRow

_f8 = ml_dtypes.float8_e4m3
_bf16 = ml_dtypes.bfloat16


def _build_nc():
    nc = bacc.Bacc(
        "TRN2", target_bir_lowering=False, debug=False, num_devices=NCORES
    )

    xq = nc.declare_dram_parameter("xq_t8", [M, S], F8, isOutput=False)
    xk = nc.declare_dram_parameter("xk_t8", [M, S], F8, isOutput=False)
    xv = nc.declare_dram_parameter("xv_t8", [M, S], F8, isOutput=False)
    wq = nc.declare_dram_parameter("wq8", [128, NMC * HG * D], F8, isOutput=False)
    wk = nc.declare_dram_parameter("wk8", [128, NMC * HG * D], F8, isOutput=False)
    wv = nc.declare_dram_parameter("wv8", [128, NMC * HG * D], F8, isOutput=False)
    wo = nc.declare_dram_parameter("wo_bf", [HG * D, M], BF, isOutput=False)
    bq = nc.declare_dram_parameter("bq", [128, 2], F32, isOutput=False)
    bk = nc.declare_dram_parameter("bk", [128, 2], F32, isOutput=False)
    bv = nc.declare_dram_parameter("bv", [1, HG * D], F32, isOutput=False)
    out_p = nc.declare_dram_parameter("out_p", [S, M], F32, isOutput=True)

    with tile.TileContext(nc) as tc:
        with (
            tc.tile_pool(name="persist", bufs=1) as pers,
            tc.tile_pool(name="work", bufs=6) as work,
            tc.tile_pool(name="ppa", bufs=1, space="PSUM") as ppa,
            tc.tile_pool(name="pps", bufs=2, space="PSUM") as pps,
            tc.tile_pool(name="ppz", bufs=3, space="PSUM") as ppz,
        ):
            # ---- persistent SBUF tensors ----
            xq_sb = pers.tile([128, NMC, S], F8, tag="xq")
            xk_sb = pers.tile([128, NMC, S], F8, tag="xk")
            xv_sb = pers.tile([128, NMC, S], F8, tag="xv")
            wq_sb = pers.tile([128, NMC, HG * D], F8, tag="wq")
            wk_sb = pers.tile([128, NMC, HG * D], F8, tag="wk")
            wv_sb = pers.tile([128, NMC, HG * D], F8, tag="wv")
            wo_sb = pers.tile([128, 2, M], BF, tag="wo")
            bq_sb = pers.tile([128, 2], F32, tag="bq")
            bk_sb = pers.tile([128, 2], F32, tag="bk")
            bv_sb = pers.tile([1, HG * D], F32, tag="bv")
            qt_sb = pers.tile([128, 2, S], BF, tag="qt")
            kt_sb = pers.tile([128, 2, S], BF, tag="kt")
            zt_sb = pers.tile([128, 2, S], BF, tag="zt")
            v_sb = pers.tile([128, NSS, HG, D + 1], BF, tag="v")
            trimask = pers.tile([128, 128], BF, tag="trimask")
            ones = pers.tile([1, SQ], F32, tag="ones")

            # ---- constants ----
            nc.gpsimd.memset(ones[:, :], 1.0)
            nc.gpsimd.memset(v_sb[:, :, :, D : D + 1], 1.0)
            # lower-triangular (inclusive) band mask: keep where row <= col
            nc.gpsimd.memset(trimask[:, :], 1.0)
            nc.gpsimd.affine_select(
                out=trimask[:, :],
                in_=trimask[:, :],
                compare_op=mybir.AluOpType.is_ge,
                fill=0.0,
                base=0,
                pattern=[[1, 128]],
                channel_multiplier=-1,
            )
            # warm the exp table set during the DMA phase
            expwarm = pers.tile([1, 1], F32, tag="expwarm")
            nc.scalar.activation(expwarm[:, :], ones[0:1, 0:1], EXP)

            # ---- input DMAs: weights/biases first, then 2 large DMAs per x ----
            nc.sync.dma_start(out=wq_sb[:, :, :], in_=wq[:, :])
            nc.sync.dma_start(out=wk_sb[:, :, :], in_=wk[:, :])
            nc.sync.dma_start(out=wv_sb[:, :, :], in_=wv[:, :])
            nc.sync.dma_start(out=bq_sb[:, :], in_=bq[:, :])
            nc.sync.dma_start(out=bk_sb[:, :], in_=bk[:, :])
            nc.sync.dma_start(out=bv_sb[:, :], in_=bv[:, :])
            for c in range(2):
                nc.sync.dma_start(out=wo_sb[:, c, :], in_=wo[128 * c : 128 * c + 128, :])
            # s-halved and ordered so proj (q,k of t=0,1) starts ~3us in
            for x_sb, x_dram, g in (
                (xq_sb, xq, 0), (xk_sb, xk, 0), (xq_sb, xq, 1), (xk_sb, xk, 1),
                (xv_sb, xv, 0), (xv_sb, xv, 1),
            ):
                nc.sync.dma_start(
                    out=x_sb[:, :, 1024 * g : 1024 * g + 1024],
                    in_=x_dram[:, 1024 * g : 1024 * g + 1024].rearrange(
                        "(c p) s -> p c s", p=128
                    ),
                )

            # ---- phase 1a: qT / kT projections ([d', s], fp8 DoubleRow) ----
            for t in range(NSQ):
                ssl = slice(SQ * t, SQ * t + SQ)
                for half in range(2):
                    dsl = slice(128 * half, 128 * half + 128)
                    for dst_sb, w_sb, x_sb, b_sb, scale in (
                        (qt_sb, wq_sb, xq_sb, bq_sb, 0.125),
                        (kt_sb, wk_sb, xk_sb, bk_sb, None),
                    ):
                        ps2p = pps.tile([128, 2, SQ], F32, tag="pps")
                        ps = ps2p[:, 0, :]
                        for mi in range(0, NMC, 2):
                            nc.tensor.matmul(
                                ps[:, :],
                                lhsT=w_sb[:, mi : mi + 2, dsl],
                                rhs=x_sb[:, mi : mi + 2, ssl],
                                start=(mi == 0),
                                stop=(mi == NMC - 2),
                                perf_mode=DR,
                            )
                        if scale is None:
                            nc.vector.tensor_scalar_add(
                                dst_sb[:, half, ssl], ps[:, :], b_sb[:, half : half + 1]
                            )
                        else:
                            nc.vector.tensor_scalar(
                                out=dst_sb[:, half, ssl],
                                in0=ps[:, :],
                                scalar1=b_sb[:, half : half + 1],
                                scalar2=scale,
                                op0=mybir.AluOpType.add,
                                op1=mybir.AluOpType.mult,
                            )

            # ---- phase 1b: v projection ([s, d'], ones col interleaved) ----
            for ss in range(NSS):
                psl = slice(128 * ss, 128 * ss + 128)
                ps2p = pps.tile([128, 2, SQ], F32, tag="pps")
                ps = ps2p[:, 0, :]
                for mi in range(0, NMC, 2):
                    nc.tensor.matmul(
                        ps[:, 0 : HG * D],
                        lhsT=xv_sb[:, mi : mi + 2, psl],
                        rhs=wv_sb[:, mi : mi + 2, :],
                        start=(mi == 0),
                        stop=False,
                        perf_mode=DR,
                    )
                nc.tensor.matmul(
                    ps[:, 0 : HG * D],
                    lhsT=ones[0:1, 0:128],
                    rhs=bv_sb[0:1, :],
                    start=False,
                    stop=True,
                )
                nc.vector.tensor_copy(
                    v_sb[:, ss, :, 0:D],
                    ps[:, 0 : HG * D].rearrange("p (g d) -> p g d", g=HG),
                )

            # ---- phase 2: attention, even/odd head pairs (row-group packed) ----
            for jq in range(NSQ):
                qsl = slice(SQ * jq, SQ * jq + SQ)
                nsk = 4 * (jq + 1)
                for c in range(2):  # head pair: heads (2c, 2c+1)
                    ps_z = [
                        ppz.tile([D + 1, SQ], F32, tag="ppz", name=f"psz{jq}_{c}_{u}")
                        for u in range(2)
                    ]
                    for si in range(nsk):
                        ksl = slice(128 * si, 128 * si + 128)
                        r = si - 4 * jq  # >=0 on diagonal-band tiles
                        w0 = 128 * r if r > 0 else 0  # fully-masked prefix
                        # both heads' scores into one 2-bank psum tile
                        ps2 = pps.tile([128, 2, SQ], F32, tag="pps")
                        for u in range(2):
                            hsl = slice(64 * u, 64 * u + 64)
                            nc.tensor.matmul(
                                ps2[:, u, w0:SQ],
                                lhsT=kt_sb[hsl, c, ksl],
                                rhs=qt_sb[hsl, c, SQ * jq + w0 : SQ * jq + SQ],
                                start=True,
                                stop=True,
                            )
                        p_bf = work.tile([128, 2, SQ], BF, tag="p")
                        nc.scalar.activation(
                            p_bf[:, :, w0:SQ], ps2[:, :, w0:SQ], EXP
                        )
                        if r >= 0:
                            for u in range(2):
                                nc.vector.tensor_mul(
                                    p_bf[:, u, w0 : w0 + 128],
                                    p_bf[:, u, w0 : w0 + 128],
                                    trimask[:, :],
                                )
                        for u in range(2):
                            h = 2 * c + u
                            nc.tensor.matmul(
                                ps_z[u][:, w0:SQ],
                                lhsT=v_sb[:, si, h, :],
                                rhs=p_bf[:, u, w0:SQ],
                                start=(si == 0),
                                stop=(si == nsk - 1),
                            )
                    # normalize both heads of the pair (~18-bit reciprocal is
                    # far below the bf16 pattern noise floor)
                    for u in range(2):
                        dn = work.tile([1, SQ], F32, tag="dn")
                        nc.vector.tensor_copy(dn[:, :], ps_z[u][D : D + 1, :])
                        recip = work.tile([1, SQ], F32, tag="recip")
                        nc.vector.reciprocal_approx_fast(
                            out=recip[:, :], in_=dn[:, :]
                        )
                        rb = work.tile([D, SQ], F32, tag="rb")
                        nc.gpsimd.partition_broadcast(rb[:, :], recip[0:1, :])
                        nc.vector.tensor_mul(
                            zt_sb[64 * u : 64 * u + 64, c, qsl],
                            ps_z[u][0:D, :],
                            rb[:, :],
                        )
                # ---- output projection for this jq's s-window ----
                for ss4 in range(4):
                    psl = slice(SQ * jq + 128 * ss4, SQ * jq + 128 * ss4 + 128)
                    for n in range(M // SQ):
                        nsl = slice(SQ * n, SQ * n + SQ)
                        if jq == NSQ - 1:
                            ps_o2 = pps.tile([128, 2, SQ], F32, tag="pps")
                            ps_o = ps_o2[:, 0, :]
                        else:
                            ps_o = ppa.tile([128, SQ], F32, tag="ppa")
                        for c in range(2):
                            nc.tensor.matmul(
                                ps_o[:, :],
                                lhsT=zt_sb[:, c, psl],
                                rhs=wo_sb[:, c, nsl],
                                start=(c == 0),
                                stop=(c == 1),
                            )
                        o_sb = work.tile([128, SQ], F32, tag="o")
                        nc.vector.tensor_copy(o_sb[:, :], ps_o[:, :])
                        nc.sync.dma_start(out=out_p[psl, nsl], in_=o_sb[:, :])

    if not nc.is_finalized():
        nc.finalize()
    return nc


_NC = None


def _get_nc():
    global _NC
    if _NC is None:
        _NC = _build_nc()
    return _NC


def _wpack(w):
    """[M, HG*D] -> partition-major [128, NMC*HG*D] (2 KiB contiguous rows)."""
    return np.ascontiguousarray(
        w.reshape(NMC, 128, HG * D).transpose(1, 0, 2).reshape(128, NMC * HG * D)
    )


def _make_in_maps(inputs):
    q8 = lambda a: np.asarray(a, np.float32).astype(_f8)
    xt = {}
    for name, key in (("xq_t8", "query_input"), ("xk_t8", "key_input"),
                      ("xv_t8", "value_input")):
        xt[name] = [np.ascontiguousarray(q8(inputs[key][b]).T) for b in range(B)]

    wq8 = q8(inputs["W_Q"])  # [H, M, D]
    wk8 = q8(inputs["W_K"])
    wv8 = q8(inputs["W_V"])
    wo = np.asarray(inputs["W_O"], np.float32)  # [H, D, M]

    in_maps = []
    for core in range(NCORES):
        b, hg = core // HG, core % HG
        hs = slice(HG * hg, HG * hg + HG)
        m = {
            "xq_t8": xt["xq_t8"][b],
            "xk_t8": xt["xk_t8"][b],
            "xv_t8": xt["xv_t8"][b],
            "wq8": _wpack(wq8[hs].transpose(1, 0, 2).reshape(M, HG * D)),
            "wk8": _wpack(wk8[hs].transpose(1, 0, 2).reshape(M, HG * D)),
            "wv8": _wpack(wv8[hs].transpose(1, 0, 2).reshape(M, HG * D)),
            "wo_bf": np.ascontiguousarray(wo[hs].reshape(HG * D, M).astype(_bf16)),
            "bq": np.ascontiguousarray(
                np.asarray(inputs["b_Q"], np.float32)[hs].reshape(2, 128).T
            ),
            "bk": np.ascontiguousarray(
                np.asarray(inputs["b_K"], np.float32)[hs].reshape(2, 128).T
            ),
            "bv": np.asarray(inputs["b_V"], np.float32)[hs].reshape(1, HG * D).copy(),
        }
        in_maps.append(m)
    return in_maps


def _run(inputs, **kw):
    nc = _get_nc()
    in_maps = _make_in_maps(inputs)
    res = run_bass_kernel_spmd(nc, in_maps, list(range(NCORES)), **kw)
    out = np.zeros((B, S, M), np.float32)
    for core in range(NCORES):
        out[core // HG] += res.results[core]["out_p"]
    out += np.asarray(inputs["b_O"], np.float32)
    return out, res


def kernel(**inputs):
    out, _ = _run(inputs)
    return out
